# revision 5
# baseline (speedup 1.0000x reference)
"""Bass/Trainium2 kernel for the single-head dense attention block.

Reference computation (per batch element b of 8):
    qkv = x @ w_qkv.T                      # [N, 3C]
    q, k, v = qkv split                    # each [N, C]
    a = softmax(q @ k.T / sqrt(C))         # [N, N]
    o = a @ v                              # [N, C]
    o2 = o.swapaxes(0,1).reshape(N, C)     # torch-faithful permutation
    out = o2 @ w_proj.T + b_proj           # [N, C]

Sharding: batch B=8 data-parallel across the 8 NeuronCores, no collectives.

Layout strategy (zero on-device transposes; host pre-transposes weights/x):
  - q.k fold:  a[n,m] = x_n^T (W_q^T W_k) x_m, so the device never computes
    q or k. Host passes hT = (W_q^T W_k)^T = W_k^T W_q in fp16; the device
    computes z = hT.T @ xT  ([c,m] layout, 1/3 the cost of qT+kT), then
    aT[m,n] = z[:,m].T @ xT[:,n].
  - v computed in [m,c] layout:      v[m,c]  = xT[:,m].T @ wvT
  - p = exp(aT/32 - ln 32) (global 1/32 downscale cancels in the softmax
    denominator; keeps p inside fp8-e4m3 range for the fp8 chunks)
  - o in [n,c] layout:               o[n,c]  = p[:,n].T @ v
    with v augmented by a ones column so rowsum(p) lands in [n,1] per-partition
  - the torch permutation satisfies out[2t+s, d] = sum_c2 o[1024s+c2, t] *
    wprojT[c2, d], i.e. proj is a plain matmul over o's partition axis in
    half-blocks; output rows written with a stride-2 row DMA.

Precision: fp16 storage everywhere (same PE speed as bf16, 3 more mantissa
bits -> base rel err ~6e-4). The first 2*F8_PAIRS m-chunks of the o matmul
run as fp8-e4m3 DoubleRow pairs (2x PE throughput); quantization error scales
as sqrt(fraction) and is kept within the 2e-2 budget.

Startup: the z phase runs nb-outer / j-mid / cc-inner with cc-granular h/x
DMAs so the first matmul only waits on ~384KB instead of ~1MB.
"""

import numpy as np

B, N, C = 8, 2048, 1024
P = 128
NB = 512          # free-dim block for matmuls (one PSUM bank)
SCALE = 1.0 / 32.0
PBIAS = float(-np.log(32.0))  # global p downscale, cancels in softmax denom
F8_PAIRS = 3      # o-matmul m-chunk pairs (of 8) in fp8 DoubleRow
MF8 = 2 * F8_PAIRS  # fp8 m-tiles (the first MF8 of 16)


def _patch_tile_drain():
    """Walrus in this container rejects >~4 sem waits on one instruction; the
    TileContext exit drain aggregates one wait per active processor. Re-emit
    them as individual SP wait_ge instructions before the drain."""
    import concourse.tile as tile
    from concourse import mybir
    from concourse.vector_clock import ScopedClock

    if getattr(tile.TileContext, "_drain_patched", False):
        return

    def _drain_and_barrier(self, tick_clock, wait_clock):
        nc = self.nc
        probe = nc.sync.nop(nofuse=True)
        wait_clock.add_sem_waits(
            probe.ins, ScopedClock({None: tick_clock.global_clock})
        )
        si = probe.ins.sync_info
        waits = list(si.on_wait) if si is not None and si.on_wait else []
        probe.ins.sync_info = mybir.SyncInfo(
            on_wait=[],
            on_update=list(si.on_update) if si is not None and si.on_update else [],
        )
        handles = {h.num: h for h in self.sems.allocated().values()}
        for w in waits:
            assert w.wait_mode == "sem-ge-imm", w
            nc.sync.wait_ge(handles[w.id], w.wait_value)
        nc.sync.drain()
        nc.all_engine_barrier()
        popped = nc._tile_sem_poison_stack.pop()
        assert popped is self._sem_poison
        nc.clear_and_free_semaphores(list(self.sems.allocated().values()))
        nc.all_engine_barrier()

    tile.TileContext._drain_and_barrier = _drain_and_barrier
    tile.TileContext._drain_patched = True


def _split_excess_waits(nc, max_keep=1):
    """Walrus in this container rejects instructions with more than a couple
    of sem waits. Move excess waits onto single-wait EventSemaphore
    instructions inserted just before the offender on the same engine
    (engines execute their stream in order, so a chain of waits == one
    multi-wait)."""
    from concourse import mybir

    ctr = 0
    for f in nc.m.functions:
        for bb in f.blocks:
            il = list(bb.instructions)
            out = []
            changed = False
            for inst in il:
                si = inst.sync_info
                waits = list(si.on_wait) if si is not None and si.on_wait else []
                if len(waits) > max_keep:
                    changed = True
                    excess, keep = waits[:-max_keep], waits[-max_keep:]
                    for w in excess:
                        ev = mybir.InstEventSemaphore(
                            name=f"I-wsplit-{ctr}", ins=[], outs=[]
                        )
                        ctr += 1
                        ev.engine = inst.engine
                        ev.sync_info = mybir.SyncInfo(on_wait=[w], on_update=[])
                        out.append(ev)
                    inst.sync_info = mybir.SyncInfo(
                        on_wait=keep,
                        on_update=list(si.on_update) if si.on_update else [],
                    )
                out.append(inst)
            if changed:
                bb.instructions = out
    return nc


def build_nc(split_waits=True):
    import concourse.bass as bass
    import concourse.tile as tile
    from concourse import mybir

    _patch_tile_drain()

    f16 = mybir.dt.float16
    e4 = mybir.dt.float8e4
    f32 = mybir.dt.float32
    DR = mybir.MatmulPerfMode.DoubleRow

    nc = bass.Bass()
    xT_ext = nc.declare_dram_parameter("xT", [C, N], f16, isOutput=False)
    hT_ext = nc.declare_dram_parameter("hT", [C, C], f16, isOutput=False)
    wvT_ext = nc.declare_dram_parameter("wvT", [C, C], f16, isOutput=False)
    wprojT_ext = nc.declare_dram_parameter("wprojT", [C, C], f16, isOutput=False)
    bias_ext = nc.declare_dram_parameter("bias", [P, C], f32, isOutput=False)
    out_ext = nc.declare_dram_parameter("out", [N, C], f32, isOutput=True)

    CC = C // P           # 8 contraction chunks over C
    MT = N // P           # 16 m-tiles
    NBLK = N // NB        # 4 n blocks
    CB = C // NB          # 2 c blocks
    M16 = MT - MF8        # fp16 m-tiles (the last M16)

    xT_r = xT_ext[:, :].rearrange("(cc p) n -> p cc n", p=P)
    hT_r = hT_ext[:, :].rearrange("(cc p) d -> p cc d", p=P)
    wvT_r = wvT_ext[:, :].rearrange("(cc p) d -> p cc d", p=P)
    wprojT_r = wprojT_ext[:, :].rearrange("(cc p) d -> p cc d", p=P)
    out_r = out_ext[:, :].rearrange("(t s) d -> t s d", s=2)

    with tile.TileContext(nc) as tc:
        with (
            tc.tile_pool(name="persist", bufs=1) as persist,
            tc.tile_pool(name="psum_main", bufs=6, space="PSUM") as psum_main,
            tc.tile_pool(name="psum_sum", bufs=2, space="PSUM") as psum_sum,
        ):
            # ---- persistent SBUF tensors ----
            z_sb = persist.tile([P, CC, N], f16, tag="z")
            v16_sb = persist.tile([P, M16, C + 1], f16, tag="v16", name="v16_sb") if M16 else None
            v8_sb = persist.tile([P, MF8, C + 1], e4, tag="v8", name="v8_sb") if MF8 else None
            wprojT_sb = persist.tile([P, CC, C], f16, tag="wprojT")
            bias_sb = persist.tile([P, C], f32, tag="bias")
            # exp bias constant (activation bias must be an AP)
            pbias_sb = persist.tile([P, 1], f32, tag="pbias")
            nc.vector.memset(pbias_sb, PBIAS)

            # ones columns for the softmax denominator
            if v16_sb is not None:
                nc.vector.memset(v16_sb[:, :, C : C + 1], 1.0)
            if v8_sb is not None:
                nc.vector.memset(v8_sb[:, :, C : C + 1], 1.0)

            def v_dst(mt, cb):
                sl = slice(cb * NB, (cb + 1) * NB)
                if mt < MF8:
                    return v8_sb[:, mt, sl]
                return v16_sb[:, mt - MF8, sl]

            # xT stays resident through phase B (aT rhs); h/wv pool is freed
            # after phase A. nb-block 0 of x and all of h arrive cc-granular
            # so the first z matmul only waits on ~384KB.
            with tc.tile_pool(name="xpool", bufs=1) as xpool:
                x0_cc = [
                    xpool.tile([P, 1, NB], f16, tag=f"x0c_{cc}", name=f"x0c_{cc}")
                    for cc in range(CC)
                ]
                x_sb = [None] + [
                    xpool.tile([P, CC, NB], f16, tag=f"xsb_{nb}", name=f"xsb_{nb}")
                    for nb in range(1, NBLK)
                ]

                def x_ap(nb, cc):
                    if nb == 0:
                        return x0_cc[cc][:, 0, :]
                    return x_sb[nb][:, cc, :]

                # ---- phase A: z = hT.T @ xT and v = xT.T @ wvT ----
                with tc.tile_pool(name="wpool", bufs=1) as wpool:
                    h_cc = [
                        wpool.tile([P, 1, C], f16, tag=f"hc_{cc}", name=f"hc_{cc}")
                        for cc in range(CC)
                    ]
                    wv_sb = [
                        wpool.tile([P, CC, NB], f16, tag=f"wvsb_{k}", name=f"wvsb_{k}")
                        for k in range(CB)
                    ]

                    # DMA order == consumption order; alternate SP/ACT rings.
                    # First the cc-interleaved h + x(nb0) pairs, then the
                    # remaining x blocks, wv, and tail-only tensors.
                    for cc in range(CC):
                        nc.sync.dma_start(out=h_cc[cc], in_=hT_r[:, cc : cc + 1, :])
                        nc.scalar.dma_start(
                            out=x0_cc[cc], in_=xT_r[:, cc : cc + 1, 0:NB]
                        )
                    nc.sync.dma_start(out=x_sb[1], in_=xT_r[:, :, NB : 2 * NB])
                    nc.scalar.dma_start(out=wv_sb[0], in_=wvT_r[:, :, 0:NB])
                    nc.sync.dma_start(out=x_sb[2], in_=xT_r[:, :, 2 * NB : 3 * NB])
                    nc.scalar.dma_start(out=wv_sb[1], in_=wvT_r[:, :, NB : 2 * NB])
                    nc.sync.dma_start(out=x_sb[3], in_=xT_r[:, :, 3 * NB : 4 * NB])
                    # needed only at the tail — load after the critical inputs
                    nc.scalar.dma_start(out=bias_sb, in_=bias_ext[:, :])
                    nc.sync.dma_start(out=wprojT_sb, in_=wprojT_r)

                    # z[j-tile, n-block] = hT[:, j].T @ xT[:, n]
                    # nb outer so the first groups only need x(nb0); cc inner
                    # so each group consumes h/x chunks in DMA arrival order.
                    for nb in range(NBLK):
                        for j in range(CC):
                            psum = psum_main.tile(
                                [P, NB], f32, tag="ps", name=f"ps_z_{nb}_{j}"
                            )
                            for cc in range(CC):
                                nc.tensor.matmul(
                                    psum,
                                    h_cc[cc][:, 0, j * P : (j + 1) * P],
                                    x_ap(nb, cc),
                                    start=(cc == 0),
                                    stop=(cc == CC - 1),
                                )
                            nc.vector.tensor_copy(
                                out=z_sb[:, j, nb * NB : (nb + 1) * NB], in_=psum
                            )

                    # v[m-tile, c-block] = xT[:, m].T @ wvT[:, c]
                    for mt in range(MT):
                        psums = [
                            psum_main.tile([P, NB], f32, tag="ps", name=f"ps_v_{mt}_{i}")
                            for i in range(CB)
                        ]
                        for cc in range(CC):
                            if mt < 4:
                                lhsT = x0_cc[cc][:, 0, mt * P : (mt + 1) * P]
                            else:
                                lhsT = x_sb[mt // 4][:, cc, (mt % 4) * P : (mt % 4 + 1) * P]
                            for cb in range(CB):
                                nc.tensor.matmul(
                                    psums[cb],
                                    lhsT,
                                    wv_sb[cb][:, cc, :],
                                    start=(cc == 0),
                                    stop=(cc == CC - 1),
                                )
                        for cb in range(CB):
                            nc.scalar.activation(
                                out=v_dst(mt, cb),
                                in_=psums[cb],
                                func=mybir.ActivationFunctionType.Copy,
                            )

                # ---- phase B: attention, one 512-wide n-block at a time ----
                with (
                    tc.tile_pool(name="attn", bufs=1) as attn_pool,
                    tc.tile_pool(name="pT", bufs=1) as pT_pool,
                    tc.tile_pool(name="small", bufs=8) as small_pool,
                    tc.tile_pool(name="outbuf", bufs=4) as out_pool,
                ):
                    o_sb = attn_pool.tile([P, MT, C], f16, tag="o")

                    def emit_proj(s):
                        # out[2t+s, d] = sum_c2 o[1024s+c2, t] wprojT[c2, d]
                        for tt in range(CC):  # 8 t-tiles of 128 (t in [0,1024))
                            psums = [
                                psum_main.tile(
                                    [P, NB], f32, tag="ps", name=f"ps_p_{s}_{tt}_{i}"
                                )
                                for i in range(CB)
                            ]
                            for k in range(CC):
                                lhsT = o_sb[:, CC * s + k, tt * P : (tt + 1) * P]
                                for db in range(CB):
                                    nc.tensor.matmul(
                                        psums[db],
                                        lhsT,
                                        wprojT_sb[:, k, db * NB : (db + 1) * NB],
                                        start=(k == 0),
                                        stop=(k == CC - 1),
                                    )
                            # chunk the very last drain so the final output
                            # DMA is small (shorter kernel tail)
                            nchunk = 4 if (s == 1 and tt == CC - 1) else 1
                            csz = NB // nchunk
                            for db in range(CB):
                                for u in range(nchunk):
                                    outt = out_pool.tile(
                                        [P, csz], f32, tag=f"outt{nchunk}",
                                        name=f"outt_{s}_{tt}_{db}_{u}",
                                    )
                                    lo = db * NB + u * csz
                                    nc.vector.tensor_add(
                                        out=outt,
                                        in0=psums[db][:, u * csz : (u + 1) * csz],
                                        in1=bias_sb[:, lo : lo + csz],
                                    )
                                    nc.sync.dma_start(
                                        out=out_r[
                                            tt * P : (tt + 1) * P, s, lo : lo + csz
                                        ],
                                        in_=outt,
                                    )

                    SB = 2  # n-blocks per superblock: one aT weight load
                    #         (z m-slice) feeds SB matmuls
                    for sbk in range(NBLK // SB):
                        pT16 = (
                            pT_pool.tile([P, M16, SB * NB], f16, tag="pT16", name="pT16")
                            if M16
                            else None
                        )
                        pT8 = (
                            pT_pool.tile([P, MF8, SB * NB], e4, tag="pT8", name="pT8")
                            if MF8
                            else None
                        )

                        def p_dst(mt, u):
                            sl = slice(u * NB, (u + 1) * NB)
                            if mt < MF8:
                                return pT8[:, mt, sl]
                            return pT16[:, mt - MF8, sl]

                        # aT[m-tile, nblk] = z[:, m].T @ xT[:, nblk]
                        # p = exp(aT/32 - ln32)
                        for mt in range(MT):
                            apsums = [
                                psum_main.tile(
                                    [P, NB], f32, tag="ps", name=f"ps_a_{sbk}_{mt}_{u}"
                                )
                                for u in range(SB)
                            ]
                            for cc in range(CC):
                                lhsT = z_sb[:, cc, mt * P : (mt + 1) * P]
                                for u in range(SB):
                                    nc.tensor.matmul(
                                        apsums[u],
                                        lhsT,
                                        x_ap(sbk * SB + u, cc),
                                        start=(cc == 0),
                                        stop=(cc == CC - 1),
                                    )
                            for u in range(SB):
                                nc.scalar.activation(
                                    out=p_dst(mt, u),
                                    in_=apsums[u],
                                    func=mybir.ActivationFunctionType.Exp,
                                    scale=SCALE,
                                    bias=pbias_sb[:, :],
                                )
                        # o[n-tile, c] = p[:, n].T @ v  (+ ones column -> rowsum)
                        # fp8 m-pairs first (DoubleRow), then fp16 m-chunks.
                        for j in range(SB * NB // P):
                            nt = sbk * (SB * NB // P) + j
                            jsl = slice(j * P, (j + 1) * P)
                            opsums = [
                                psum_main.tile([P, NB], f32, tag="ps", name=f"ps_o_{nt}_{i}")
                                for i in range(CB)
                            ]
                            osum = psum_sum.tile([P, 1], f32, tag="ps_sum", name=f"ps_sum_{nt}")
                            nmm = F8_PAIRS + M16  # instructions per cb chain
                            idx = 0
                            for q in range(F8_PAIRS):
                                psl = slice(2 * q, 2 * q + 2)
                                for cb in range(CB):
                                    nc.tensor.matmul(
                                        opsums[cb],
                                        pT8[:, psl, jsl],
                                        v8_sb[:, psl, cb * NB : (cb + 1) * NB],
                                        start=(idx == 0),
                                        stop=(idx == nmm - 1),
                                        perf_mode=DR,
                                    )
                                nc.tensor.matmul(
                                    osum,
                                    pT8[:, psl, jsl],
                                    v8_sb[:, psl, C : C + 1],
                                    start=(idx == 0),
                                    stop=(idx == nmm - 1),
                                    perf_mode=DR,
                                )
                                idx += 1
                            for mt in range(M16):
                                lhsT = pT16[:, mt, jsl]
                                for cb in range(CB):
                                    nc.tensor.matmul(
                                        opsums[cb],
                                        lhsT,
                                        v16_sb[:, mt, cb * NB : (cb + 1) * NB],
                                        start=(idx == 0),
                                        stop=(idx == nmm - 1),
                                    )
                                nc.tensor.matmul(
                                    osum,
                                    lhsT,
                                    v16_sb[:, mt, C : C + 1],
                                    start=(idx == 0),
                                    stop=(idx == nmm - 1),
                                )
                                idx += 1
                            recip = small_pool.tile([P, 1], f32, tag="recip")
                            nc.vector.reciprocal(out=recip, in_=osum)
                            for cb in range(CB):
                                nc.vector.tensor_scalar_mul(
                                    out=o_sb[:, nt, cb * NB : (cb + 1) * NB],
                                    in0=opsums[cb],
                                    scalar1=recip,
                                )
                        # phase C half s=sbk: its o-tiles (nt 0..7 for s=0,
                        # 8..15 for s=1) are exactly this superblock's output,
                        # so the proj matmuls + output DMAs interleave here.
                        emit_proj(sbk)
    if split_waits:
        _split_excess_waits(nc)
    return nc


_CACHED_NC = None


def _get_nc():
    global _CACHED_NC
    if _CACHED_NC is None:
        _CACHED_NC = build_nc()
    return _CACHED_NC


def _make_in_maps(x, w_qkv, w_proj, b_proj):
    x = np.asarray(x, dtype=np.float32)
    w_qkv = np.asarray(w_qkv, dtype=np.float32)
    w_proj = np.asarray(w_proj, dtype=np.float32)
    b_proj = np.asarray(b_proj, dtype=np.float32)

    w_q, w_k, w_v = w_qkv[0:C], w_qkv[C : 2 * C], w_qkv[2 * C : 3 * C]
    # hT = (W_q^T W_k)^T = W_k^T W_q, computed in f32 then rounded once
    hT = np.ascontiguousarray(w_k.T @ w_q).astype(np.float16)
    wvT = np.ascontiguousarray(w_v.T).astype(np.float16)
    wprojT = np.ascontiguousarray(w_proj.T).astype(np.float16)
    bias = np.ascontiguousarray(np.broadcast_to(b_proj, (P, C)))
    in_maps = []
    for b in range(B):
        xT = np.ascontiguousarray(x[b].T).astype(np.float16)
        in_maps.append(
            {"xT": xT, "hT": hT, "wvT": wvT, "wprojT": wprojT, "bias": bias}
        )
    return in_maps


def kernel(x, w_qkv, w_proj, b_proj):
    from concourse.bass_utils import run_bass_kernel_spmd

    nc = _get_nc()
    in_maps = _make_in_maps(x, w_qkv, w_proj, b_proj)
    res = run_bass_kernel_spmd(nc, in_maps, core_ids=list(range(B)))
    return np.stack([res.results[b]["out"] for b in range(B)], axis=0)


def kernel_traced(x, w_qkv, w_proj, b_proj, **trace_kwargs):
    """Like kernel() but with NTFF profiling; returns (out, BassKernelResults)."""
    from concourse.bass_utils import run_bass_kernel_spmd

    nc = _get_nc()
    in_maps = _make_in_maps(x, w_qkv, w_proj, b_proj)
    res = run_bass_kernel_spmd(
        nc, in_maps, core_ids=list(range(B)), trace=True, **trace_kwargs
    )
    out = np.stack([res.results[b]["out"] for b in range(B)], axis=0)
    return out, res


# revision 6
# speedup vs baseline: 1.1925x; 1.1925x over previous
"""Bass/Trainium2 kernel for the single-head dense attention block.

Reference computation (per batch element b of 8):
    qkv = x @ w_qkv.T                      # [N, 3C]
    q, k, v = qkv split                    # each [N, C]
    a = softmax(q @ k.T / sqrt(C))         # [N, N]
    o = a @ v                              # [N, C]
    o2 = o.swapaxes(0,1).reshape(N, C)     # torch-faithful permutation
    out = o2 @ w_proj.T + b_proj           # [N, C]

Sharding: batch B=8 data-parallel across the 8 NeuronCores, no collectives.

Layout strategy (zero on-device transposes; host pre-transposes weights/x):
  - q.k fold:  a[n,m] = x_n^T (W_q^T W_k) x_m, so the device never computes
    q or k. Host passes hT = (W_q^T W_k)^T = W_k^T W_q in bf16; the device
    computes z = hT.T @ xT  ([c,m] layout, 1/3 the cost of qT+kT), then
    aT[m,n] = z[:,m].T @ xT[:,n].
  - v computed in [m,c] layout:      v[m,c]  = xT[:,m].T @ wvT
  - p = exp(aT/32 - ln 32) (global 1/32 downscale cancels in the softmax
    denominator; keeps p inside fp8-e4m3 range for the fp8 chunks)
  - o in [n,c] layout:               o[n,c]  = p[:,n].T @ v
    with v augmented by a ones column so rowsum(p) lands in [n,1] per-partition
  - the torch permutation satisfies out[2t+s, d] = sum_c2 o[1024s+c2, t] *
    wprojT[c2, d], i.e. proj is a plain matmul over o's partition axis in
    half-blocks; output rows written with a stride-2 row DMA.

Precision: bf16 storage (measured: fp16 runs 1.2x slower on the real PE). The first 2*F8_PAIRS m-chunks of the o matmul
run as fp8-e4m3 DoubleRow pairs (2x PE throughput); quantization error scales
as sqrt(fraction) and is kept within the 2e-2 budget.

Startup: the z phase runs nb-outer / j-mid / cc-inner with cc-granular h/x
DMAs so the first matmul only waits on ~384KB instead of ~1MB.
"""

import numpy as np
import ml_dtypes

bf16h = ml_dtypes.bfloat16

B, N, C = 8, 2048, 1024
P = 128
NB = 512          # free-dim block for matmuls (one PSUM bank)
SCALE = 1.0 / 32.0
PBIAS = float(-np.log(32.0))  # global p downscale, cancels in softmax denom
F8_PAIRS = 3      # o-matmul m-chunk pairs (of 8) in fp8 DoubleRow
MF8 = 2 * F8_PAIRS  # fp8 m-tiles (the first MF8 of 16)


def _patch_tile_drain():
    """Walrus in this container rejects >~4 sem waits on one instruction; the
    TileContext exit drain aggregates one wait per active processor. Re-emit
    them as individual SP wait_ge instructions before the drain."""
    import concourse.tile as tile
    from concourse import mybir
    from concourse.vector_clock import ScopedClock

    if getattr(tile.TileContext, "_drain_patched", False):
        return

    def _drain_and_barrier(self, tick_clock, wait_clock):
        nc = self.nc
        probe = nc.sync.nop(nofuse=True)
        wait_clock.add_sem_waits(
            probe.ins, ScopedClock({None: tick_clock.global_clock})
        )
        si = probe.ins.sync_info
        waits = list(si.on_wait) if si is not None and si.on_wait else []
        probe.ins.sync_info = mybir.SyncInfo(
            on_wait=[],
            on_update=list(si.on_update) if si is not None and si.on_update else [],
        )
        handles = {h.num: h for h in self.sems.allocated().values()}
        for w in waits:
            assert w.wait_mode == "sem-ge-imm", w
            nc.sync.wait_ge(handles[w.id], w.wait_value)
        nc.sync.drain()
        nc.all_engine_barrier()
        popped = nc._tile_sem_poison_stack.pop()
        assert popped is self._sem_poison
        nc.clear_and_free_semaphores(list(self.sems.allocated().values()))
        nc.all_engine_barrier()

    tile.TileContext._drain_and_barrier = _drain_and_barrier
    tile.TileContext._drain_patched = True


def _split_excess_waits(nc, max_keep=1):
    """Walrus in this container rejects instructions with more than a couple
    of sem waits. Move excess waits onto single-wait EventSemaphore
    instructions inserted just before the offender on the same engine
    (engines execute their stream in order, so a chain of waits == one
    multi-wait)."""
    from concourse import mybir

    ctr = 0
    for f in nc.m.functions:
        for bb in f.blocks:
            il = list(bb.instructions)
            out = []
            changed = False
            for inst in il:
                si = inst.sync_info
                waits = list(si.on_wait) if si is not None and si.on_wait else []
                if len(waits) > max_keep:
                    changed = True
                    excess, keep = waits[:-max_keep], waits[-max_keep:]
                    for w in excess:
                        ev = mybir.InstEventSemaphore(
                            name=f"I-wsplit-{ctr}", ins=[], outs=[]
                        )
                        ctr += 1
                        ev.engine = inst.engine
                        ev.sync_info = mybir.SyncInfo(on_wait=[w], on_update=[])
                        out.append(ev)
                    inst.sync_info = mybir.SyncInfo(
                        on_wait=keep,
                        on_update=list(si.on_update) if si.on_update else [],
                    )
                out.append(inst)
            if changed:
                bb.instructions = out
    return nc


def build_nc(split_waits=True):
    import concourse.bass as bass
    import concourse.tile as tile
    from concourse import mybir

    _patch_tile_drain()

    f16 = mybir.dt.bfloat16  # bf16: real PE runs fp16 1.2x slower than bf16
    e4 = mybir.dt.float8e4
    f32 = mybir.dt.float32
    DR = mybir.MatmulPerfMode.DoubleRow

    nc = bass.Bass()
    xT_ext = nc.declare_dram_parameter("xT", [C, N], f16, isOutput=False)
    hT_ext = nc.declare_dram_parameter("hT", [C, C], f16, isOutput=False)
    wvT_ext = nc.declare_dram_parameter("wvT", [C, C], f16, isOutput=False)
    wprojT_ext = nc.declare_dram_parameter("wprojT", [C, C], f16, isOutput=False)
    bias_ext = nc.declare_dram_parameter("bias", [P, C], f32, isOutput=False)
    out_ext = nc.declare_dram_parameter("out", [N, C], f32, isOutput=True)

    CC = C // P           # 8 contraction chunks over C
    MT = N // P           # 16 m-tiles
    NBLK = N // NB        # 4 n blocks
    CB = C // NB          # 2 c blocks
    M16 = MT - MF8        # fp16 m-tiles (the last M16)

    xT_r = xT_ext[:, :].rearrange("(cc p) n -> p cc n", p=P)
    hT_r = hT_ext[:, :].rearrange("(cc p) d -> p cc d", p=P)
    wvT_r = wvT_ext[:, :].rearrange("(cc p) d -> p cc d", p=P)
    wprojT_r = wprojT_ext[:, :].rearrange("(cc p) d -> p cc d", p=P)
    out_r = out_ext[:, :].rearrange("(t s) d -> t s d", s=2)

    with tile.TileContext(nc) as tc:
        with (
            tc.tile_pool(name="persist", bufs=1) as persist,
            tc.tile_pool(name="psum_main", bufs=6, space="PSUM") as psum_main,
            tc.tile_pool(name="psum_sum", bufs=2, space="PSUM") as psum_sum,
        ):
            # ---- persistent SBUF tensors ----
            z_sb = persist.tile([P, CC, N], f16, tag="z")
            v16_sb = persist.tile([P, M16, C + 1], f16, tag="v16", name="v16_sb") if M16 else None
            v8_sb = persist.tile([P, MF8, C + 16], e4, tag="v8", name="v8_sb") if MF8 else None
            wprojT_sb = persist.tile([P, CC, C], f16, tag="wprojT")
            bias_sb = persist.tile([P, C], f32, tag="bias")
            # exp bias constant (activation bias must be an AP)
            pbias_sb = persist.tile([P, 1], f32, tag="pbias")
            nc.vector.memset(pbias_sb, PBIAS)

            # ones columns for the softmax denominator
            if v16_sb is not None:
                nc.vector.memset(v16_sb[:, :, C : C + 1], 1.0)
            if v8_sb is not None:
                nc.vector.memset(v8_sb[:, :, C : C + 1], 1.0)

            def v_dst(mt, cb):
                sl = slice(cb * NB, (cb + 1) * NB)
                if mt < MF8:
                    return v8_sb[:, mt, sl]
                return v16_sb[:, mt - MF8, sl]

            # xT stays resident through phase B (aT rhs); h/wv pool is freed
            # after phase A. nb-block 0 of x and all of h arrive cc-granular
            # so the first z matmul only waits on ~384KB.
            with tc.tile_pool(name="xpool", bufs=1) as xpool:
                x0_cc = [
                    xpool.tile([P, 1, NB], f16, tag=f"x0c_{cc}", name=f"x0c_{cc}")
                    for cc in range(CC)
                ]
                x_sb = [None] + [
                    xpool.tile([P, CC, NB], f16, tag=f"xsb_{nb}", name=f"xsb_{nb}")
                    for nb in range(1, NBLK)
                ]

                def x_ap(nb, cc):
                    if nb == 0:
                        return x0_cc[cc][:, 0, :]
                    return x_sb[nb][:, cc, :]

                # ---- phase A: z = hT.T @ xT and v = xT.T @ wvT ----
                with tc.tile_pool(name="wpool", bufs=1) as wpool:
                    h_cc = [
                        wpool.tile([P, 1, C], f16, tag=f"hc_{cc}", name=f"hc_{cc}")
                        for cc in range(CC)
                    ]
                    wv_sb = [
                        wpool.tile([P, CC, NB], f16, tag=f"wvsb_{k}", name=f"wvsb_{k}")
                        for k in range(CB)
                    ]

                    # DMA order == consumption order; alternate SP/ACT rings.
                    # First the cc-interleaved h + x(nb0) pairs, then the
                    # remaining x blocks, wv, and tail-only tensors.
                    for cc in range(CC):
                        nc.sync.dma_start(out=h_cc[cc], in_=hT_r[:, cc : cc + 1, :])
                        nc.scalar.dma_start(
                            out=x0_cc[cc], in_=xT_r[:, cc : cc + 1, 0:NB]
                        )
                    nc.sync.dma_start(out=x_sb[1], in_=xT_r[:, :, NB : 2 * NB])
                    nc.scalar.dma_start(out=wv_sb[0], in_=wvT_r[:, :, 0:NB])
                    nc.sync.dma_start(out=x_sb[2], in_=xT_r[:, :, 2 * NB : 3 * NB])
                    nc.scalar.dma_start(out=wv_sb[1], in_=wvT_r[:, :, NB : 2 * NB])
                    nc.sync.dma_start(out=x_sb[3], in_=xT_r[:, :, 3 * NB : 4 * NB])
                    # needed only at the tail — load after the critical inputs
                    nc.scalar.dma_start(out=bias_sb, in_=bias_ext[:, :])
                    nc.sync.dma_start(out=wprojT_sb, in_=wprojT_r)

                    # z[j-tile, n-block] = hT[:, j].T @ xT[:, n]
                    # nb outer so the first groups only need x(nb0); cc inner
                    # so each group consumes h/x chunks in DMA arrival order.
                    for nb in range(NBLK):
                        for j in range(CC):
                            psum = psum_main.tile(
                                [P, NB], f32, tag="ps", name=f"ps_z_{nb}_{j}"
                            )
                            for cc in range(CC):
                                nc.tensor.matmul(
                                    psum,
                                    h_cc[cc][:, 0, j * P : (j + 1) * P],
                                    x_ap(nb, cc),
                                    start=(cc == 0),
                                    stop=(cc == CC - 1),
                                )
                            nc.vector.tensor_copy(
                                out=z_sb[:, j, nb * NB : (nb + 1) * NB], in_=psum
                            )

                    # v[m-tile, c-block] = xT[:, m].T @ wvT[:, c]
                    for mt in range(MT):
                        psums = [
                            psum_main.tile([P, NB], f32, tag="ps", name=f"ps_v_{mt}_{i}")
                            for i in range(CB)
                        ]
                        for cc in range(CC):
                            if mt < 4:
                                lhsT = x0_cc[cc][:, 0, mt * P : (mt + 1) * P]
                            else:
                                lhsT = x_sb[mt // 4][:, cc, (mt % 4) * P : (mt % 4 + 1) * P]
                            for cb in range(CB):
                                nc.tensor.matmul(
                                    psums[cb],
                                    lhsT,
                                    wv_sb[cb][:, cc, :],
                                    start=(cc == 0),
                                    stop=(cc == CC - 1),
                                )
                        for cb in range(CB):
                            nc.scalar.activation(
                                out=v_dst(mt, cb),
                                in_=psums[cb],
                                func=mybir.ActivationFunctionType.Copy,
                            )

                # ---- phase B: attention, one 512-wide n-block at a time ----
                with (
                    tc.tile_pool(name="attn", bufs=1) as attn_pool,
                    tc.tile_pool(name="pT", bufs=1) as pT_pool,
                    tc.tile_pool(name="small", bufs=8) as small_pool,
                    tc.tile_pool(name="outbuf", bufs=4) as out_pool,
                ):
                    o_sb = attn_pool.tile([P, MT, C], f16, tag="o")

                    def emit_proj(s):
                        # out[2t+s, d] = sum_c2 o[1024s+c2, t] wprojT[c2, d]
                        for tt in range(CC):  # 8 t-tiles of 128 (t in [0,1024))
                            psums = [
                                psum_main.tile(
                                    [P, NB], f32, tag="ps", name=f"ps_p_{s}_{tt}_{i}"
                                )
                                for i in range(CB)
                            ]
                            for k in range(CC):
                                lhsT = o_sb[:, CC * s + k, tt * P : (tt + 1) * P]
                                for db in range(CB):
                                    nc.tensor.matmul(
                                        psums[db],
                                        lhsT,
                                        wprojT_sb[:, k, db * NB : (db + 1) * NB],
                                        start=(k == 0),
                                        stop=(k == CC - 1),
                                    )
                            # chunk the very last drain so the final output
                            # DMA is small (shorter kernel tail)
                            nchunk = 4 if (s == 1 and tt == CC - 1) else 1
                            csz = NB // nchunk
                            for db in range(CB):
                                for u in range(nchunk):
                                    outt = out_pool.tile(
                                        [P, csz], f32, tag=f"outt{nchunk}",
                                        name=f"outt_{s}_{tt}_{db}_{u}",
                                    )
                                    lo = db * NB + u * csz
                                    nc.vector.tensor_add(
                                        out=outt,
                                        in0=psums[db][:, u * csz : (u + 1) * csz],
                                        in1=bias_sb[:, lo : lo + csz],
                                    )
                                    nc.sync.dma_start(
                                        out=out_r[
                                            tt * P : (tt + 1) * P, s, lo : lo + csz
                                        ],
                                        in_=outt,
                                    )

                    SB = 2  # n-blocks per superblock: one aT weight load
                    #         (z m-slice) feeds SB matmuls
                    for sbk in range(NBLK // SB):
                        pT16 = (
                            pT_pool.tile([P, M16, SB * NB], f16, tag="pT16", name="pT16")
                            if M16
                            else None
                        )
                        pT8 = (
                            pT_pool.tile([P, MF8, SB * NB], e4, tag="pT8", name="pT8")
                            if MF8
                            else None
                        )

                        def p_dst(mt, u):
                            sl = slice(u * NB, (u + 1) * NB)
                            if mt < MF8:
                                return pT8[:, mt, sl]
                            return pT16[:, mt - MF8, sl]

                        # aT[m-tile, nblk] = z[:, m].T @ xT[:, nblk]
                        # p = exp(aT/32 - ln32)
                        for mt in range(MT):
                            apsums = [
                                psum_main.tile(
                                    [P, NB], f32, tag="ps", name=f"ps_a_{sbk}_{mt}_{u}"
                                )
                                for u in range(SB)
                            ]
                            for cc in range(CC):
                                lhsT = z_sb[:, cc, mt * P : (mt + 1) * P]
                                for u in range(SB):
                                    nc.tensor.matmul(
                                        apsums[u],
                                        lhsT,
                                        x_ap(sbk * SB + u, cc),
                                        start=(cc == 0),
                                        stop=(cc == CC - 1),
                                    )
                            for u in range(SB):
                                nc.scalar.activation(
                                    out=p_dst(mt, u),
                                    in_=apsums[u],
                                    func=mybir.ActivationFunctionType.Exp,
                                    scale=SCALE,
                                    bias=pbias_sb[:, :],
                                )
                        # o[n-tile, c] = p[:, n].T @ v  (+ ones column -> rowsum)
                        # fp8 m-pairs first (DoubleRow), then fp16 m-chunks.
                        for j in range(SB * NB // P):
                            nt = sbk * (SB * NB // P) + j
                            jsl = slice(j * P, (j + 1) * P)
                            opsums = [
                                psum_main.tile([P, NB], f32, tag="ps", name=f"ps_o_{nt}_{i}")
                                for i in range(CB)
                            ]
                            osum = psum_sum.tile([P, 1], f32, tag="ps_sum", name=f"ps_sum_{nt}")
                            nmm = F8_PAIRS + M16  # instructions per cb chain
                            idx = 0
                            for q in range(F8_PAIRS):
                                psl = slice(2 * q, 2 * q + 2)
                                for cb in range(CB):
                                    nc.tensor.matmul(
                                        opsums[cb],
                                        pT8[:, psl, jsl],
                                        v8_sb[:, psl, cb * NB : (cb + 1) * NB],
                                        start=(idx == 0),
                                        stop=(idx == nmm - 1),
                                        perf_mode=DR,
                                    )
                                nc.tensor.matmul(
                                    osum,
                                    pT8[:, psl, jsl],
                                    v8_sb[:, psl, C : C + 1],
                                    start=(idx == 0),
                                    stop=(idx == nmm - 1),
                                    perf_mode=DR,
                                )
                                idx += 1
                            for mt in range(M16):
                                lhsT = pT16[:, mt, jsl]
                                for cb in range(CB):
                                    nc.tensor.matmul(
                                        opsums[cb],
                                        lhsT,
                                        v16_sb[:, mt, cb * NB : (cb + 1) * NB],
                                        start=(idx == 0),
                                        stop=(idx == nmm - 1),
                                    )
                                nc.tensor.matmul(
                                    osum,
                                    lhsT,
                                    v16_sb[:, mt, C : C + 1],
                                    start=(idx == 0),
                                    stop=(idx == nmm - 1),
                                )
                                idx += 1
                            recip = small_pool.tile([P, 1], f32, tag="recip")
                            nc.vector.reciprocal(out=recip, in_=osum)
                            for cb in range(CB):
                                nc.vector.tensor_scalar_mul(
                                    out=o_sb[:, nt, cb * NB : (cb + 1) * NB],
                                    in0=opsums[cb],
                                    scalar1=recip,
                                )
                        # phase C half s=sbk: its o-tiles (nt 0..7 for s=0,
                        # 8..15 for s=1) are exactly this superblock's output,
                        # so the proj matmuls + output DMAs interleave here.
                        emit_proj(sbk)
    if split_waits:
        _split_excess_waits(nc)
    return nc


_CACHED_NC = None


def _get_nc():
    global _CACHED_NC
    if _CACHED_NC is None:
        _CACHED_NC = build_nc()
    return _CACHED_NC


def _make_in_maps(x, w_qkv, w_proj, b_proj):
    x = np.asarray(x, dtype=np.float32)
    w_qkv = np.asarray(w_qkv, dtype=np.float32)
    w_proj = np.asarray(w_proj, dtype=np.float32)
    b_proj = np.asarray(b_proj, dtype=np.float32)

    w_q, w_k, w_v = w_qkv[0:C], w_qkv[C : 2 * C], w_qkv[2 * C : 3 * C]
    # hT = (W_q^T W_k)^T = W_k^T W_q, computed in f32 then rounded once
    hT = np.ascontiguousarray(w_k.T @ w_q).astype(bf16h)
    wvT = np.ascontiguousarray(w_v.T).astype(bf16h)
    wprojT = np.ascontiguousarray(w_proj.T).astype(bf16h)
    bias = np.ascontiguousarray(np.broadcast_to(b_proj, (P, C)))
    in_maps = []
    for b in range(B):
        xT = np.ascontiguousarray(x[b].T).astype(bf16h)
        in_maps.append(
            {"xT": xT, "hT": hT, "wvT": wvT, "wprojT": wprojT, "bias": bias}
        )
    return in_maps


def kernel(x, w_qkv, w_proj, b_proj):
    from concourse.bass_utils import run_bass_kernel_spmd

    nc = _get_nc()
    in_maps = _make_in_maps(x, w_qkv, w_proj, b_proj)
    res = run_bass_kernel_spmd(nc, in_maps, core_ids=list(range(B)))
    return np.stack([res.results[b]["out"] for b in range(B)], axis=0)


def kernel_traced(x, w_qkv, w_proj, b_proj, **trace_kwargs):
    """Like kernel() but with NTFF profiling; returns (out, BassKernelResults)."""
    from concourse.bass_utils import run_bass_kernel_spmd

    nc = _get_nc()
    in_maps = _make_in_maps(x, w_qkv, w_proj, b_proj)
    res = run_bass_kernel_spmd(
        nc, in_maps, core_ids=list(range(B)), trace=True, **trace_kwargs
    )
    out = np.stack([res.results[b]["out"] for b in range(B)], axis=0)
    return out, res


# revision 9
# speedup vs baseline: 1.2023x; 1.0083x over previous
"""Bass/Trainium2 kernel for the single-head dense attention block.

Reference computation (per batch element b of 8):
    qkv = x @ w_qkv.T                      # [N, 3C]
    q, k, v = qkv split                    # each [N, C]
    a = softmax(q @ k.T / sqrt(C))         # [N, N]
    o = a @ v                              # [N, C]
    o2 = o.swapaxes(0,1).reshape(N, C)     # torch-faithful permutation
    out = o2 @ w_proj.T + b_proj           # [N, C]

Sharding: batch B=8 data-parallel across the 8 NeuronCores, no collectives.

Layout strategy (zero on-device transposes; host pre-transposes weights/x):
  - q.k fold:  a[n,m] = x_n^T (W_q^T W_k) x_m, so the device never computes
    q or k. Host passes hT = (W_q^T W_k)^T = W_k^T W_q in bf16; the device
    computes z = hT.T @ xT  ([c,m] layout, 1/3 the cost of qT+kT), then
    aT[m,n] = z[:,m].T @ xT[:,n].
  - v computed in [m,c] layout:      v[m,c]  = xT[:,m].T @ wvT
  - p = exp(aT/32 - ln 32) (global 1/32 downscale cancels in the softmax
    denominator; keeps p inside fp8-e4m3 range for the fp8 chunks)
  - o in [n,c] layout:               o[n,c]  = p[:,n].T @ v
    with v augmented by a ones column so rowsum(p) lands in [n,1] per-partition
  - the torch permutation satisfies out[2t+s, d] = sum_c2 o[1024s+c2, t] *
    wprojT[c2, d], i.e. proj is a plain matmul over o's partition axis in
    half-blocks; output rows written with a stride-2 row DMA.

Precision: bf16 storage (measured: fp16 runs 1.2x slower on the real PE). The first 2*F8_PAIRS m-chunks of the o matmul
run as fp8-e4m3 DoubleRow pairs (2x PE throughput); quantization error scales
as sqrt(fraction) and is kept within the 2e-2 budget.

Startup: the z phase runs nb-outer / j-mid / cc-inner with cc-granular h/x
DMAs so the first matmul only waits on ~384KB instead of ~1MB.
"""

import numpy as np
import ml_dtypes

bf16h = ml_dtypes.bfloat16

B, N, C = 8, 2048, 1024
P = 128
NB = 512          # free-dim block for matmuls (one PSUM bank)
SCALE = 1.0 / 32.0
PBIAS = float(-np.log(32.0))  # global p downscale, cancels in softmax denom
F8_PAIRS = 3      # o-matmul m-chunk pairs (of 8) in fp8 DoubleRow
MF8 = 2 * F8_PAIRS  # fp8 m-tiles (the first MF8 of 16)


def _patch_tile_drain():
    """Walrus in this container rejects >~4 sem waits on one instruction; the
    TileContext exit drain aggregates one wait per active processor. Re-emit
    them as individual SP wait_ge instructions before the drain."""
    import concourse.tile as tile
    from concourse import mybir
    from concourse.vector_clock import ScopedClock

    if getattr(tile.TileContext, "_drain_patched", False):
        return

    def _drain_and_barrier(self, tick_clock, wait_clock):
        nc = self.nc
        probe = nc.sync.nop(nofuse=True)
        wait_clock.add_sem_waits(
            probe.ins, ScopedClock({None: tick_clock.global_clock})
        )
        si = probe.ins.sync_info
        waits = list(si.on_wait) if si is not None and si.on_wait else []
        probe.ins.sync_info = mybir.SyncInfo(
            on_wait=[],
            on_update=list(si.on_update) if si is not None and si.on_update else [],
        )
        handles = {h.num: h for h in self.sems.allocated().values()}
        for w in waits:
            assert w.wait_mode == "sem-ge-imm", w
            nc.sync.wait_ge(handles[w.id], w.wait_value)
        nc.sync.drain()
        nc.all_engine_barrier()
        popped = nc._tile_sem_poison_stack.pop()
        assert popped is self._sem_poison
        nc.clear_and_free_semaphores(list(self.sems.allocated().values()))
        nc.all_engine_barrier()

    tile.TileContext._drain_and_barrier = _drain_and_barrier
    tile.TileContext._drain_patched = True


def _split_excess_waits(nc, max_keep=1):
    """Walrus in this container rejects instructions with more than a couple
    of sem waits. Move excess waits onto single-wait EventSemaphore
    instructions inserted just before the offender on the same engine
    (engines execute their stream in order, so a chain of waits == one
    multi-wait)."""
    from concourse import mybir

    ctr = 0
    for f in nc.m.functions:
        for bb in f.blocks:
            il = list(bb.instructions)
            out = []
            changed = False
            for inst in il:
                si = inst.sync_info
                waits = list(si.on_wait) if si is not None and si.on_wait else []
                if len(waits) > max_keep:
                    changed = True
                    excess, keep = waits[:-max_keep], waits[-max_keep:]
                    for w in excess:
                        ev = mybir.InstEventSemaphore(
                            name=f"I-wsplit-{ctr}", ins=[], outs=[]
                        )
                        ctr += 1
                        ev.engine = inst.engine
                        ev.sync_info = mybir.SyncInfo(on_wait=[w], on_update=[])
                        out.append(ev)
                    inst.sync_info = mybir.SyncInfo(
                        on_wait=keep,
                        on_update=list(si.on_update) if si.on_update else [],
                    )
                out.append(inst)
            if changed:
                bb.instructions = out
    return nc


def build_nc(split_waits=True):
    import concourse.bass as bass
    import concourse.tile as tile
    from concourse import mybir

    _patch_tile_drain()

    f16 = mybir.dt.bfloat16  # bf16: real PE runs fp16 1.2x slower than bf16
    e4 = mybir.dt.float8e4
    f32 = mybir.dt.float32
    DR = mybir.MatmulPerfMode.DoubleRow

    nc = bass.Bass()
    xT_ext = nc.declare_dram_parameter("xT", [C, N], f16, isOutput=False)
    hT_ext = nc.declare_dram_parameter("hT", [C, C], f16, isOutput=False)
    wvT_ext = nc.declare_dram_parameter("wvT", [C, C], f16, isOutput=False)
    wprojT_ext = nc.declare_dram_parameter("wprojT", [C, C], f16, isOutput=False)
    bias_ext = nc.declare_dram_parameter("bias", [P, C], f32, isOutput=False)
    out_ext = nc.declare_dram_parameter("out", [N, C], f32, isOutput=True)

    CC = C // P           # 8 contraction chunks over C
    MT = N // P           # 16 m-tiles
    NBLK = N // NB        # 4 n blocks
    CB = C // NB          # 2 c blocks
    M16 = MT - MF8        # fp16 m-tiles (the last M16)

    xT_r = xT_ext[:, :].rearrange("(cc p) n -> p cc n", p=P)
    hT_r = hT_ext[:, :].rearrange("(cc p) d -> p cc d", p=P)
    wvT_r = wvT_ext[:, :].rearrange("(cc p) d -> p cc d", p=P)
    wprojT_r = wprojT_ext[:, :].rearrange("(cc p) d -> p cc d", p=P)
    out_r = out_ext[:, :].rearrange("(t s) d -> t s d", s=2)

    with tile.TileContext(nc) as tc:
        with (
            tc.tile_pool(name="persist", bufs=1) as persist,
            tc.tile_pool(name="psum_main", bufs=6, space="PSUM") as psum_main,
            tc.tile_pool(name="psum_sum", bufs=2, space="PSUM") as psum_sum,
        ):
            # ---- persistent SBUF tensors ----
            z_sb = persist.tile([P, CC, N], f16, tag="z")
            v16_sb = persist.tile([P, M16, C + 1], f16, tag="v16", name="v16_sb") if M16 else None
            v8_sb = persist.tile([P, MF8, C + 16], e4, tag="v8", name="v8_sb") if MF8 else None
            wprojT_sb = persist.tile([P, CC, C], f16, tag="wprojT")
            bias_sb = persist.tile([P, C], f32, tag="bias")
            # exp bias constant (activation bias must be an AP)
            pbias_sb = persist.tile([P, 1], f32, tag="pbias")
            nc.vector.memset(pbias_sb, PBIAS)

            # ones columns for the softmax denominator
            if v16_sb is not None:
                nc.vector.memset(v16_sb[:, :, C : C + 1], 1.0)
            if v8_sb is not None:
                nc.vector.memset(v8_sb[:, :, C : C + 1], 1.0)

            def v_dst(mt, cb):
                sl = slice(cb * NB, (cb + 1) * NB)
                if mt < MF8:
                    return v8_sb[:, mt, sl]
                return v16_sb[:, mt - MF8, sl]

            # xT stays resident through phase B (aT rhs); h/wv pool is freed
            # after phase A. nb-block 0 of x and all of h arrive cc-granular
            # so the first z matmul only waits on ~384KB.
            with tc.tile_pool(name="xpool", bufs=1) as xpool:
                x0_cc = [
                    xpool.tile([P, 1, NB], f16, tag=f"x0c_{cc}", name=f"x0c_{cc}")
                    for cc in range(CC)
                ]
                x_sb = [None] + [
                    xpool.tile([P, CC, NB], f16, tag=f"xsb_{nb}", name=f"xsb_{nb}")
                    for nb in range(1, NBLK)
                ]

                def x_ap(nb, cc):
                    if nb == 0:
                        return x0_cc[cc][:, 0, :]
                    return x_sb[nb][:, cc, :]

                # ---- phase A: z = hT.T @ xT and v = xT.T @ wvT ----
                with tc.tile_pool(name="wpool", bufs=1) as wpool:
                    h_cc = [
                        wpool.tile([P, 1, C], f16, tag=f"hc_{cc}", name=f"hc_{cc}")
                        for cc in range(CC)
                    ]
                    wv_sb = [
                        wpool.tile([P, CC, NB], f16, tag=f"wvsb_{k}", name=f"wvsb_{k}")
                        for k in range(CB)
                    ]

                    # DMA order == consumption order; alternate SP/ACT rings.
                    # First the cc-interleaved h + x(nb0) pairs, then the
                    # remaining x blocks, wv, and tail-only tensors.
                    for cc in range(CC):
                        nc.sync.dma_start(out=h_cc[cc], in_=hT_r[:, cc : cc + 1, :])
                        nc.scalar.dma_start(
                            out=x0_cc[cc], in_=xT_r[:, cc : cc + 1, 0:NB]
                        )
                    nc.sync.dma_start(out=x_sb[1], in_=xT_r[:, :, NB : 2 * NB])
                    nc.scalar.dma_start(out=wv_sb[0], in_=wvT_r[:, :, 0:NB])
                    nc.sync.dma_start(out=x_sb[2], in_=xT_r[:, :, 2 * NB : 3 * NB])
                    nc.scalar.dma_start(out=wv_sb[1], in_=wvT_r[:, :, NB : 2 * NB])
                    nc.sync.dma_start(out=x_sb[3], in_=xT_r[:, :, 3 * NB : 4 * NB])
                    # needed only at the tail — load after the critical inputs
                    nc.scalar.dma_start(out=bias_sb, in_=bias_ext[:, :])
                    nc.sync.dma_start(out=wprojT_sb, in_=wprojT_r)

                    # z[j-tile, n-block] = hT[:, j].T @ xT[:, n]
                    # nb outer so the first groups only need x(nb0); cc inner
                    # so each group consumes h/x chunks in DMA arrival order.
                    for nb in range(NBLK):
                        for j in range(CC):
                            psum = psum_main.tile(
                                [P, NB], f32, tag="ps", name=f"ps_z_{nb}_{j}"
                            )
                            for cc in range(CC):
                                nc.tensor.matmul(
                                    psum,
                                    h_cc[cc][:, 0, j * P : (j + 1) * P],
                                    x_ap(nb, cc),
                                    start=(cc == 0),
                                    stop=(cc == CC - 1),
                                )
                            nc.vector.tensor_copy(
                                out=z_sb[:, j, nb * NB : (nb + 1) * NB], in_=psum
                            )

                    # v[m-tile, c-block] = xT[:, m].T @ wvT[:, c]
                    for mt in range(MT):
                        psums = [
                            psum_main.tile([P, NB], f32, tag="ps", name=f"ps_v_{mt}_{i}")
                            for i in range(CB)
                        ]
                        for cc in range(CC):
                            if mt < 4:
                                lhsT = x0_cc[cc][:, 0, mt * P : (mt + 1) * P]
                            else:
                                lhsT = x_sb[mt // 4][:, cc, (mt % 4) * P : (mt % 4 + 1) * P]
                            for cb in range(CB):
                                nc.tensor.matmul(
                                    psums[cb],
                                    lhsT,
                                    wv_sb[cb][:, cc, :],
                                    start=(cc == 0),
                                    stop=(cc == CC - 1),
                                )
                        for cb in range(CB):
                            nc.scalar.activation(
                                out=v_dst(mt, cb),
                                in_=psums[cb],
                                func=mybir.ActivationFunctionType.Copy,
                            )

                # ---- phase B: attention, one 512-wide n-block at a time ----
                with (
                    tc.tile_pool(name="attn", bufs=1) as attn_pool,
                    tc.tile_pool(name="pT", bufs=1) as pT_pool,
                    tc.tile_pool(name="small", bufs=8) as small_pool,
                    tc.tile_pool(name="outbuf", bufs=4) as out_pool,
                ):
                    o_sb = attn_pool.tile([P, MT, C], f16, tag="o")

                    def emit_proj(s):
                        # out[2t+s, d] = sum_c2 o[1024s+c2, t] wprojT[c2, d]
                        for tt in range(CC):  # 8 t-tiles of 128 (t in [0,1024))
                            psums = [
                                psum_main.tile(
                                    [P, NB], f32, tag="ps", name=f"ps_p_{s}_{tt}_{i}"
                                )
                                for i in range(CB)
                            ]
                            for k in range(CC):
                                lhsT = o_sb[:, CC * s + k, tt * P : (tt + 1) * P]
                                for db in range(CB):
                                    nc.tensor.matmul(
                                        psums[db],
                                        lhsT,
                                        wprojT_sb[:, k, db * NB : (db + 1) * NB],
                                        start=(k == 0),
                                        stop=(k == CC - 1),
                                    )
                            # chunk the very last drain (2 halves per psum,
                            # DMAs spread over idle queues) for a short tail
                            nchunk = 2 if (s == 1 and tt == CC - 1) else 1
                            csz = NB // nchunk
                            dma_engs = [nc.sync, nc.scalar, nc.gpsimd, nc.sync]
                            di = 0
                            for db in range(CB):
                                for u in range(nchunk):
                                    outt = out_pool.tile(
                                        [P, csz], f32, tag=f"outt{nchunk}",
                                        name=f"outt_{s}_{tt}_{db}_{u}",
                                    )
                                    lo = db * NB + u * csz
                                    nc.vector.tensor_add(
                                        out=outt,
                                        in0=psums[db][:, u * csz : (u + 1) * csz],
                                        in1=bias_sb[:, lo : lo + csz],
                                    )
                                    eng = dma_engs[di % len(dma_engs)] if nchunk > 1 else nc.sync
                                    di += 1
                                    eng.dma_start(
                                        out=out_r[
                                            tt * P : (tt + 1) * P, s, lo : lo + csz
                                        ],
                                        in_=outt,
                                    )

                    SB = 2  # n-blocks per superblock: one aT weight load
                    #         (z m-slice) feeds SB matmuls
                    for sbk in range(NBLK // SB):
                        pT16 = (
                            pT_pool.tile([P, M16, SB * NB], f16, tag="pT16", name="pT16")
                            if M16
                            else None
                        )
                        pT8 = (
                            pT_pool.tile([P, MF8, SB * NB], e4, tag="pT8", name="pT8")
                            if MF8
                            else None
                        )

                        def p_dst(mt, u):
                            sl = slice(u * NB, (u + 1) * NB)
                            if mt < MF8:
                                return pT8[:, mt, sl]
                            return pT16[:, mt - MF8, sl]

                        # aT[m-tile, nblk] = z[:, m].T @ xT[:, nblk]
                        # p = exp(aT/32 - ln32)
                        for mt in range(MT):
                            apsums = [
                                psum_main.tile(
                                    [P, NB], f32, tag="ps", name=f"ps_a_{sbk}_{mt}_{u}"
                                )
                                for u in range(SB)
                            ]
                            for cc in range(CC):
                                lhsT = z_sb[:, cc, mt * P : (mt + 1) * P]
                                for u in range(SB):
                                    nc.tensor.matmul(
                                        apsums[u],
                                        lhsT,
                                        x_ap(sbk * SB + u, cc),
                                        start=(cc == 0),
                                        stop=(cc == CC - 1),
                                    )
                            for u in range(SB):
                                nc.scalar.activation(
                                    out=p_dst(mt, u),
                                    in_=apsums[u],
                                    func=mybir.ActivationFunctionType.Exp,
                                    scale=SCALE,
                                    bias=pbias_sb[:, :],
                                )
                        # o[n-tile, c] = p[:, n].T @ v  (+ ones column -> rowsum)
                        # fp8 m-pairs first (DoubleRow), then fp16 m-chunks.
                        for j in range(SB * NB // P):
                            nt = sbk * (SB * NB // P) + j
                            jsl = slice(j * P, (j + 1) * P)
                            opsums = [
                                psum_main.tile([P, NB], f32, tag="ps", name=f"ps_o_{nt}_{i}")
                                for i in range(CB)
                            ]
                            osum = psum_sum.tile([P, 1], f32, tag="ps_sum", name=f"ps_sum_{nt}")
                            nmm = F8_PAIRS + M16  # instructions per cb chain
                            idx = 0
                            # order per chunk: cb0, osum, cb1 — the 1-col osum
                            # runs under the already-loaded stationary, and the
                            # next weight load hides under the long cb1/cb0.
                            for q in range(F8_PAIRS):
                                psl = slice(2 * q, 2 * q + 2)
                                nc.tensor.matmul(
                                    opsums[0],
                                    pT8[:, psl, jsl],
                                    v8_sb[:, psl, 0:NB],
                                    start=(idx == 0),
                                    stop=(idx == nmm - 1),
                                    perf_mode=DR,
                                )
                                nc.tensor.matmul(
                                    osum,
                                    pT8[:, psl, jsl],
                                    v8_sb[:, psl, C : C + 1],
                                    start=(idx == 0),
                                    stop=(idx == nmm - 1),
                                    perf_mode=DR,
                                )
                                nc.tensor.matmul(
                                    opsums[1],
                                    pT8[:, psl, jsl],
                                    v8_sb[:, psl, NB : 2 * NB],
                                    start=(idx == 0),
                                    stop=(idx == nmm - 1),
                                    perf_mode=DR,
                                )
                                idx += 1
                            for mt in range(M16):
                                lhsT = pT16[:, mt, jsl]
                                nc.tensor.matmul(
                                    opsums[0],
                                    lhsT,
                                    v16_sb[:, mt, 0:NB],
                                    start=(idx == 0),
                                    stop=(idx == nmm - 1),
                                )
                                nc.tensor.matmul(
                                    osum,
                                    lhsT,
                                    v16_sb[:, mt, C : C + 1],
                                    start=(idx == 0),
                                    stop=(idx == nmm - 1),
                                )
                                nc.tensor.matmul(
                                    opsums[1],
                                    lhsT,
                                    v16_sb[:, mt, NB : 2 * NB],
                                    start=(idx == 0),
                                    stop=(idx == nmm - 1),
                                )
                                idx += 1
                            recip = small_pool.tile([P, 1], f32, tag="recip")
                            nc.vector.reciprocal(out=recip, in_=osum)
                            for cb in range(CB):
                                nc.vector.tensor_scalar_mul(
                                    out=o_sb[:, nt, cb * NB : (cb + 1) * NB],
                                    in0=opsums[cb],
                                    scalar1=recip,
                                )
                        # phase C half s=sbk: its o-tiles (nt 0..7 for s=0,
                        # 8..15 for s=1) are exactly this superblock's output,
                        # so the proj matmuls + output DMAs interleave here.
                        emit_proj(sbk)
    if split_waits:
        _split_excess_waits(nc)
    return nc


_CACHED_NC = None


def _get_nc():
    global _CACHED_NC
    if _CACHED_NC is None:
        _CACHED_NC = build_nc()
    return _CACHED_NC


def _make_in_maps(x, w_qkv, w_proj, b_proj):
    x = np.asarray(x, dtype=np.float32)
    w_qkv = np.asarray(w_qkv, dtype=np.float32)
    w_proj = np.asarray(w_proj, dtype=np.float32)
    b_proj = np.asarray(b_proj, dtype=np.float32)

    w_q, w_k, w_v = w_qkv[0:C], w_qkv[C : 2 * C], w_qkv[2 * C : 3 * C]
    # hT = (W_q^T W_k)^T = W_k^T W_q, computed in f32 then rounded once
    hT = np.ascontiguousarray(w_k.T @ w_q).astype(bf16h)
    wvT = np.ascontiguousarray(w_v.T).astype(bf16h)
    wprojT = np.ascontiguousarray(w_proj.T).astype(bf16h)
    bias = np.ascontiguousarray(np.broadcast_to(b_proj, (P, C)))
    in_maps = []
    for b in range(B):
        xT = np.ascontiguousarray(x[b].T).astype(bf16h)
        in_maps.append(
            {"xT": xT, "hT": hT, "wvT": wvT, "wprojT": wprojT, "bias": bias}
        )
    return in_maps


def kernel(x, w_qkv, w_proj, b_proj):
    from concourse.bass_utils import run_bass_kernel_spmd

    nc = _get_nc()
    in_maps = _make_in_maps(x, w_qkv, w_proj, b_proj)
    res = run_bass_kernel_spmd(nc, in_maps, core_ids=list(range(B)))
    return np.stack([res.results[b]["out"] for b in range(B)], axis=0)


def kernel_traced(x, w_qkv, w_proj, b_proj, **trace_kwargs):
    """Like kernel() but with NTFF profiling; returns (out, BassKernelResults)."""
    from concourse.bass_utils import run_bass_kernel_spmd

    nc = _get_nc()
    in_maps = _make_in_maps(x, w_qkv, w_proj, b_proj)
    res = run_bass_kernel_spmd(
        nc, in_maps, core_ids=list(range(B)), trace=True, **trace_kwargs
    )
    out = np.stack([res.results[b]["out"] for b in range(B)], axis=0)
    return out, res


# revision 10
# speedup vs baseline: 1.2126x; 1.0085x over previous
"""Bass/Trainium2 kernel for the single-head dense attention block.

Reference computation (per batch element b of 8):
    qkv = x @ w_qkv.T                      # [N, 3C]
    q, k, v = qkv split                    # each [N, C]
    a = softmax(q @ k.T / sqrt(C))         # [N, N]
    o = a @ v                              # [N, C]
    o2 = o.swapaxes(0,1).reshape(N, C)     # torch-faithful permutation
    out = o2 @ w_proj.T + b_proj           # [N, C]

Sharding: batch B=8 data-parallel across the 8 NeuronCores, no collectives.

Layout strategy (zero on-device transposes; host pre-transposes weights/x):
  - q.k fold:  a[n,m] = x_n^T (W_q^T W_k) x_m, so the device never computes
    q or k. Host passes hT = (W_q^T W_k)^T = W_k^T W_q in bf16; the device
    computes z = hT.T @ xT  ([c,m] layout, 1/3 the cost of qT+kT), then
    aT[m,n] = z[:,m].T @ xT[:,n].
  - v computed in [m,c] layout:      v[m,c]  = xT[:,m].T @ wvT
  - p = exp(aT/32 - ln 32) (global 1/32 downscale cancels in the softmax
    denominator; keeps p inside fp8-e4m3 range for the fp8 chunks)
  - o in [n,c] layout:               o[n,c]  = p[:,n].T @ v
    with v augmented by a ones column so rowsum(p) lands in [n,1] per-partition
  - the torch permutation satisfies out[2t+s, d] = sum_c2 o[1024s+c2, t] *
    wprojT[c2, d], i.e. proj is a plain matmul over o's partition axis in
    half-blocks; output rows written with a stride-2 row DMA.

Precision: bf16 storage (measured: fp16 runs 1.2x slower on the real PE). The first 2*F8_PAIRS m-chunks of the o matmul
run as fp8-e4m3 DoubleRow pairs (2x PE throughput); quantization error scales
as sqrt(fraction) and is kept within the 2e-2 budget.

Startup: the z phase runs nb-outer / j-mid / cc-inner with cc-granular h/x
DMAs so the first matmul only waits on ~384KB instead of ~1MB.
"""

import numpy as np
import ml_dtypes

bf16h = ml_dtypes.bfloat16

B, N, C = 8, 2048, 1024
P = 128
NB = 512          # free-dim block for matmuls (one PSUM bank)
SCALE = 1.0 / 32.0
PBIAS = float(-np.log(32.0))  # global p downscale, cancels in softmax denom
F8_PAIRS = 3      # o-matmul m-chunk pairs (of 8) in fp8 DoubleRow
MF8 = 2 * F8_PAIRS  # fp8 m-tiles (the first MF8 of 16)


def _patch_tile_drain():
    """Walrus in this container rejects >~4 sem waits on one instruction; the
    TileContext exit drain aggregates one wait per active processor. Re-emit
    them as individual SP wait_ge instructions before the drain."""
    import concourse.tile as tile
    from concourse import mybir
    from concourse.vector_clock import ScopedClock

    if getattr(tile.TileContext, "_drain_patched", False):
        return

    def _drain_and_barrier(self, tick_clock, wait_clock):
        nc = self.nc
        probe = nc.sync.nop(nofuse=True)
        wait_clock.add_sem_waits(
            probe.ins, ScopedClock({None: tick_clock.global_clock})
        )
        si = probe.ins.sync_info
        waits = list(si.on_wait) if si is not None and si.on_wait else []
        probe.ins.sync_info = mybir.SyncInfo(
            on_wait=[],
            on_update=list(si.on_update) if si is not None and si.on_update else [],
        )
        handles = {h.num: h for h in self.sems.allocated().values()}
        for w in waits:
            assert w.wait_mode == "sem-ge-imm", w
            nc.sync.wait_ge(handles[w.id], w.wait_value)
        nc.sync.drain()
        nc.all_engine_barrier()
        popped = nc._tile_sem_poison_stack.pop()
        assert popped is self._sem_poison
        nc.clear_and_free_semaphores(list(self.sems.allocated().values()))
        nc.all_engine_barrier()

    tile.TileContext._drain_and_barrier = _drain_and_barrier
    tile.TileContext._drain_patched = True


def _split_excess_waits(nc, max_keep=1):
    """Walrus in this container rejects instructions with more than a couple
    of sem waits. Move excess waits onto single-wait EventSemaphore
    instructions inserted just before the offender on the same engine
    (engines execute their stream in order, so a chain of waits == one
    multi-wait)."""
    from concourse import mybir

    ctr = 0
    for f in nc.m.functions:
        for bb in f.blocks:
            il = list(bb.instructions)
            out = []
            changed = False
            for inst in il:
                si = inst.sync_info
                waits = list(si.on_wait) if si is not None and si.on_wait else []
                if len(waits) > max_keep:
                    changed = True
                    excess, keep = waits[:-max_keep], waits[-max_keep:]
                    for w in excess:
                        ev = mybir.InstEventSemaphore(
                            name=f"I-wsplit-{ctr}", ins=[], outs=[]
                        )
                        ctr += 1
                        ev.engine = inst.engine
                        ev.sync_info = mybir.SyncInfo(on_wait=[w], on_update=[])
                        out.append(ev)
                    inst.sync_info = mybir.SyncInfo(
                        on_wait=keep,
                        on_update=list(si.on_update) if si.on_update else [],
                    )
                out.append(inst)
            if changed:
                bb.instructions = out
    return nc


def build_nc(split_waits=True):
    import concourse.bass as bass
    import concourse.tile as tile
    from concourse import mybir

    _patch_tile_drain()

    f16 = mybir.dt.bfloat16  # bf16: real PE runs fp16 1.2x slower than bf16
    e4 = mybir.dt.float8e4
    f32 = mybir.dt.float32
    DR = mybir.MatmulPerfMode.DoubleRow

    nc = bass.Bass()
    xT_ext = nc.declare_dram_parameter("xT", [C, N], f16, isOutput=False)
    hT_ext = nc.declare_dram_parameter("hT", [C, C], f16, isOutput=False)
    wvT_ext = nc.declare_dram_parameter("wvT", [C, C], f16, isOutput=False)
    wprojT_ext = nc.declare_dram_parameter("wprojT", [C, C], f16, isOutput=False)
    bias_ext = nc.declare_dram_parameter("bias", [P, C], f32, isOutput=False)
    out_ext = nc.declare_dram_parameter("out", [N, C], f32, isOutput=True)

    CC = C // P           # 8 contraction chunks over C
    MT = N // P           # 16 m-tiles
    NBLK = N // NB        # 4 n blocks
    CB = C // NB          # 2 c blocks
    M16 = MT - MF8        # fp16 m-tiles (the last M16)

    xT_r = xT_ext[:, :].rearrange("(cc p) n -> p cc n", p=P)
    hT_r = hT_ext[:, :].rearrange("(cc p) d -> p cc d", p=P)
    wvT_r = wvT_ext[:, :].rearrange("(cc p) d -> p cc d", p=P)
    wprojT_r = wprojT_ext[:, :].rearrange("(cc p) d -> p cc d", p=P)
    out_r = out_ext[:, :].rearrange("(t s) d -> t s d", s=2)

    with tile.TileContext(nc) as tc:
        with (
            tc.tile_pool(name="persist", bufs=1) as persist,
            tc.tile_pool(name="psum_main", bufs=6, space="PSUM") as psum_main,
        ):
            # ---- persistent SBUF tensors ----
            z_sb = persist.tile([P, CC, N], f16, tag="z")
            v16_sb = persist.tile([P, M16, C + 1], f16, tag="v16", name="v16_sb") if M16 else None
            v8_sb = persist.tile([P, MF8, C + 16], e4, tag="v8", name="v8_sb") if MF8 else None
            wprojT_sb = persist.tile([P, CC, C], f16, tag="wprojT")
            bias_sb = persist.tile([P, C], f32, tag="bias")
            # exp bias constant (activation bias must be an AP)
            pbias_sb = persist.tile([P, 1], f32, tag="pbias")
            nc.vector.memset(pbias_sb, PBIAS)

            # ones columns for the softmax denominator
            if v16_sb is not None:
                nc.vector.memset(v16_sb[:, :, C : C + 1], 1.0)
            if v8_sb is not None:
                nc.vector.memset(v8_sb[:, :, C : C + 1], 1.0)

            def v_dst(mt, cb):
                sl = slice(cb * NB, (cb + 1) * NB)
                if mt < MF8:
                    return v8_sb[:, mt, sl]
                return v16_sb[:, mt - MF8, sl]

            # xT stays resident through phase B (aT rhs); h/wv pool is freed
            # after phase A. nb-block 0 of x and all of h arrive cc-granular
            # so the first z matmul only waits on ~384KB.
            with tc.tile_pool(name="xpool", bufs=1) as xpool:
                x0_cc = [
                    xpool.tile([P, 1, NB], f16, tag=f"x0c_{cc}", name=f"x0c_{cc}")
                    for cc in range(CC)
                ]
                x_sb = [None] + [
                    xpool.tile([P, CC, NB], f16, tag=f"xsb_{nb}", name=f"xsb_{nb}")
                    for nb in range(1, NBLK)
                ]

                def x_ap(nb, cc):
                    if nb == 0:
                        return x0_cc[cc][:, 0, :]
                    return x_sb[nb][:, cc, :]

                # ---- phase A: z = hT.T @ xT and v = xT.T @ wvT ----
                with (
                    tc.tile_pool(name="wpool", bufs=1) as wpool,
                    tc.tile_pool(name="psz", bufs=2, space="PSUM") as psz,
                ):
                    h_cc = [
                        wpool.tile([P, 1, C], f16, tag=f"hc_{cc}", name=f"hc_{cc}")
                        for cc in range(CC)
                    ]
                    wv_sb = [
                        wpool.tile([P, CC, NB], f16, tag=f"wvsb_{k}", name=f"wvsb_{k}")
                        for k in range(CB)
                    ]

                    # DMA order == consumption order; alternate SP/ACT rings.
                    # First the cc-interleaved h + x(nb0) pairs, then the
                    # remaining x blocks, wv, and tail-only tensors.
                    for cc in range(CC):
                        nc.sync.dma_start(out=h_cc[cc], in_=hT_r[:, cc : cc + 1, :])
                        nc.scalar.dma_start(
                            out=x0_cc[cc], in_=xT_r[:, cc : cc + 1, 0:NB]
                        )
                    nc.sync.dma_start(out=x_sb[1], in_=xT_r[:, :, NB : 2 * NB])
                    nc.scalar.dma_start(out=wv_sb[0], in_=wvT_r[:, :, 0:NB])
                    nc.sync.dma_start(out=x_sb[2], in_=xT_r[:, :, 2 * NB : 3 * NB])
                    nc.scalar.dma_start(out=wv_sb[1], in_=wvT_r[:, :, NB : 2 * NB])
                    nc.sync.dma_start(out=x_sb[3], in_=xT_r[:, :, 3 * NB : 4 * NB])
                    # needed only at the tail — load after the critical inputs
                    nc.scalar.dma_start(out=bias_sb, in_=bias_ext[:, :])
                    nc.sync.dma_start(out=wprojT_sb, in_=wprojT_r)

                    # z[j-tile, n-block] = hT[:, j].T @ xT[:, n]
                    # cc OUTER across 8 concurrently-open PSUM banks: the PE
                    # consumes h/x chunks in exact DMA arrival order, so the
                    # stream never stalls on a chunk that is still in flight.
                    for nb in range(NBLK):
                        zpsums = [
                            psum_main.tile([P, NB], f32, tag="ps", name=f"ps_z_{nb}_{j}")
                            for j in range(6)
                        ] + [
                            psz.tile([P, NB], f32, tag="psz", name=f"psz_{nb}_{j}")
                            for j in range(2)
                        ]
                        for cc in range(CC):
                            for j in range(CC):
                                nc.tensor.matmul(
                                    zpsums[j],
                                    h_cc[cc][:, 0, j * P : (j + 1) * P],
                                    x_ap(nb, cc),
                                    start=(cc == 0),
                                    stop=(cc == CC - 1),
                                )
                        for j in range(CC):
                            eng = nc.vector if j % 2 == 0 else nc.scalar
                            if j % 2 == 0:
                                nc.vector.tensor_copy(
                                    out=z_sb[:, j, nb * NB : (nb + 1) * NB],
                                    in_=zpsums[j],
                                )
                            else:
                                nc.scalar.activation(
                                    out=z_sb[:, j, nb * NB : (nb + 1) * NB],
                                    in_=zpsums[j],
                                    func=mybir.ActivationFunctionType.Copy,
                                )

                    # v[m-tile, c-block] = xT[:, m].T @ wvT[:, c]
                    for mt in range(MT):
                        psums = [
                            psum_main.tile([P, NB], f32, tag="ps", name=f"ps_v_{mt}_{i}")
                            for i in range(CB)
                        ]
                        for cc in range(CC):
                            if mt < 4:
                                lhsT = x0_cc[cc][:, 0, mt * P : (mt + 1) * P]
                            else:
                                lhsT = x_sb[mt // 4][:, cc, (mt % 4) * P : (mt % 4 + 1) * P]
                            for cb in range(CB):
                                nc.tensor.matmul(
                                    psums[cb],
                                    lhsT,
                                    wv_sb[cb][:, cc, :],
                                    start=(cc == 0),
                                    stop=(cc == CC - 1),
                                )
                        for cb in range(CB):
                            nc.scalar.activation(
                                out=v_dst(mt, cb),
                                in_=psums[cb],
                                func=mybir.ActivationFunctionType.Copy,
                            )

                # ---- phase B: attention, one 512-wide n-block at a time ----
                with (
                    tc.tile_pool(name="attn", bufs=1) as attn_pool,
                    tc.tile_pool(name="pT", bufs=1) as pT_pool,
                    tc.tile_pool(name="small", bufs=8) as small_pool,
                    tc.tile_pool(name="outbuf", bufs=4) as out_pool,
                    tc.tile_pool(name="psum_sum", bufs=2, space="PSUM") as psum_sum,
                ):
                    o_sb = attn_pool.tile([P, MT, C], f16, tag="o")

                    def emit_proj(s):
                        # out[2t+s, d] = sum_c2 o[1024s+c2, t] wprojT[c2, d]
                        for tt in range(CC):  # 8 t-tiles of 128 (t in [0,1024))
                            psums = [
                                psum_main.tile(
                                    [P, NB], f32, tag="ps", name=f"ps_p_{s}_{tt}_{i}"
                                )
                                for i in range(CB)
                            ]
                            for k in range(CC):
                                lhsT = o_sb[:, CC * s + k, tt * P : (tt + 1) * P]
                                for db in range(CB):
                                    nc.tensor.matmul(
                                        psums[db],
                                        lhsT,
                                        wprojT_sb[:, k, db * NB : (db + 1) * NB],
                                        start=(k == 0),
                                        stop=(k == CC - 1),
                                    )
                            # chunk the very last drain (2 halves per psum,
                            # DMAs spread over idle queues) for a short tail
                            nchunk = 2 if (s == 1 and tt == CC - 1) else 1
                            csz = NB // nchunk
                            dma_engs = [nc.sync, nc.scalar, nc.gpsimd, nc.sync]
                            di = 0
                            for db in range(CB):
                                for u in range(nchunk):
                                    outt = out_pool.tile(
                                        [P, csz], f32, tag=f"outt{nchunk}",
                                        name=f"outt_{s}_{tt}_{db}_{u}",
                                    )
                                    lo = db * NB + u * csz
                                    nc.vector.tensor_add(
                                        out=outt,
                                        in0=psums[db][:, u * csz : (u + 1) * csz],
                                        in1=bias_sb[:, lo : lo + csz],
                                    )
                                    eng = dma_engs[di % len(dma_engs)] if nchunk > 1 else nc.sync
                                    di += 1
                                    eng.dma_start(
                                        out=out_r[
                                            tt * P : (tt + 1) * P, s, lo : lo + csz
                                        ],
                                        in_=outt,
                                    )

                    SB = 2  # n-blocks per superblock: one aT weight load
                    #         (z m-slice) feeds SB matmuls
                    for sbk in range(NBLK // SB):
                        pT16 = (
                            pT_pool.tile([P, M16, SB * NB], f16, tag="pT16", name="pT16")
                            if M16
                            else None
                        )
                        pT8 = (
                            pT_pool.tile([P, MF8, SB * NB], e4, tag="pT8", name="pT8")
                            if MF8
                            else None
                        )

                        def p_dst(mt, u):
                            sl = slice(u * NB, (u + 1) * NB)
                            if mt < MF8:
                                return pT8[:, mt, sl]
                            return pT16[:, mt - MF8, sl]

                        # aT[m-tile, nblk] = z[:, m].T @ xT[:, nblk]
                        # p = exp(aT/32 - ln32)
                        for mt in range(MT):
                            apsums = [
                                psum_main.tile(
                                    [P, NB], f32, tag="ps", name=f"ps_a_{sbk}_{mt}_{u}"
                                )
                                for u in range(SB)
                            ]
                            for cc in range(CC):
                                lhsT = z_sb[:, cc, mt * P : (mt + 1) * P]
                                for u in range(SB):
                                    nc.tensor.matmul(
                                        apsums[u],
                                        lhsT,
                                        x_ap(sbk * SB + u, cc),
                                        start=(cc == 0),
                                        stop=(cc == CC - 1),
                                    )
                            for u in range(SB):
                                nc.scalar.activation(
                                    out=p_dst(mt, u),
                                    in_=apsums[u],
                                    func=mybir.ActivationFunctionType.Exp,
                                    scale=SCALE,
                                    bias=pbias_sb[:, :],
                                )
                        # o[n-tile, c] = p[:, n].T @ v  (+ ones column -> rowsum)
                        # fp8 m-pairs first (DoubleRow), then fp16 m-chunks.
                        for j in range(SB * NB // P):
                            nt = sbk * (SB * NB // P) + j
                            jsl = slice(j * P, (j + 1) * P)
                            opsums = [
                                psum_main.tile([P, NB], f32, tag="ps", name=f"ps_o_{nt}_{i}")
                                for i in range(CB)
                            ]
                            osum = psum_sum.tile([P, 1], f32, tag="ps_sum", name=f"ps_sum_{nt}")
                            nmm = F8_PAIRS + M16  # instructions per cb chain
                            idx = 0
                            # order per chunk: cb0, osum, cb1 — the 1-col osum
                            # runs under the already-loaded stationary, and the
                            # next weight load hides under the long cb1/cb0.
                            for q in range(F8_PAIRS):
                                psl = slice(2 * q, 2 * q + 2)
                                nc.tensor.matmul(
                                    opsums[0],
                                    pT8[:, psl, jsl],
                                    v8_sb[:, psl, 0:NB],
                                    start=(idx == 0),
                                    stop=(idx == nmm - 1),
                                    perf_mode=DR,
                                )
                                nc.tensor.matmul(
                                    osum,
                                    pT8[:, psl, jsl],
                                    v8_sb[:, psl, C : C + 1],
                                    start=(idx == 0),
                                    stop=(idx == nmm - 1),
                                    perf_mode=DR,
                                )
                                nc.tensor.matmul(
                                    opsums[1],
                                    pT8[:, psl, jsl],
                                    v8_sb[:, psl, NB : 2 * NB],
                                    start=(idx == 0),
                                    stop=(idx == nmm - 1),
                                    perf_mode=DR,
                                )
                                idx += 1
                            for mt in range(M16):
                                lhsT = pT16[:, mt, jsl]
                                nc.tensor.matmul(
                                    opsums[0],
                                    lhsT,
                                    v16_sb[:, mt, 0:NB],
                                    start=(idx == 0),
                                    stop=(idx == nmm - 1),
                                )
                                nc.tensor.matmul(
                                    osum,
                                    lhsT,
                                    v16_sb[:, mt, C : C + 1],
                                    start=(idx == 0),
                                    stop=(idx == nmm - 1),
                                )
                                nc.tensor.matmul(
                                    opsums[1],
                                    lhsT,
                                    v16_sb[:, mt, NB : 2 * NB],
                                    start=(idx == 0),
                                    stop=(idx == nmm - 1),
                                )
                                idx += 1
                            recip = small_pool.tile([P, 1], f32, tag="recip")
                            nc.vector.reciprocal(out=recip, in_=osum)
                            for cb in range(CB):
                                nc.vector.tensor_scalar_mul(
                                    out=o_sb[:, nt, cb * NB : (cb + 1) * NB],
                                    in0=opsums[cb],
                                    scalar1=recip,
                                )
                        # phase C half s=sbk: its o-tiles (nt 0..7 for s=0,
                        # 8..15 for s=1) are exactly this superblock's output,
                        # so the proj matmuls + output DMAs interleave here.
                        emit_proj(sbk)
    if split_waits:
        _split_excess_waits(nc)
    return nc


_CACHED_NC = None


def _get_nc():
    global _CACHED_NC
    if _CACHED_NC is None:
        _CACHED_NC = build_nc()
    return _CACHED_NC


def _make_in_maps(x, w_qkv, w_proj, b_proj):
    x = np.asarray(x, dtype=np.float32)
    w_qkv = np.asarray(w_qkv, dtype=np.float32)
    w_proj = np.asarray(w_proj, dtype=np.float32)
    b_proj = np.asarray(b_proj, dtype=np.float32)

    w_q, w_k, w_v = w_qkv[0:C], w_qkv[C : 2 * C], w_qkv[2 * C : 3 * C]
    # hT = (W_q^T W_k)^T = W_k^T W_q, computed in f32 then rounded once
    hT = np.ascontiguousarray(w_k.T @ w_q).astype(bf16h)
    wvT = np.ascontiguousarray(w_v.T).astype(bf16h)
    wprojT = np.ascontiguousarray(w_proj.T).astype(bf16h)
    bias = np.ascontiguousarray(np.broadcast_to(b_proj, (P, C)))
    in_maps = []
    for b in range(B):
        xT = np.ascontiguousarray(x[b].T).astype(bf16h)
        in_maps.append(
            {"xT": xT, "hT": hT, "wvT": wvT, "wprojT": wprojT, "bias": bias}
        )
    return in_maps


def kernel(x, w_qkv, w_proj, b_proj):
    from concourse.bass_utils import run_bass_kernel_spmd

    nc = _get_nc()
    in_maps = _make_in_maps(x, w_qkv, w_proj, b_proj)
    res = run_bass_kernel_spmd(nc, in_maps, core_ids=list(range(B)))
    return np.stack([res.results[b]["out"] for b in range(B)], axis=0)


def kernel_traced(x, w_qkv, w_proj, b_proj, **trace_kwargs):
    """Like kernel() but with NTFF profiling; returns (out, BassKernelResults)."""
    from concourse.bass_utils import run_bass_kernel_spmd

    nc = _get_nc()
    in_maps = _make_in_maps(x, w_qkv, w_proj, b_proj)
    res = run_bass_kernel_spmd(
        nc, in_maps, core_ids=list(range(B)), trace=True, **trace_kwargs
    )
    out = np.stack([res.results[b]["out"] for b in range(B)], axis=0)
    return out, res


# revision 12
# speedup vs baseline: 1.2230x; 1.0086x over previous
"""Bass/Trainium2 kernel for the single-head dense attention block.

Reference computation (per batch element b of 8):
    qkv = x @ w_qkv.T                      # [N, 3C]
    q, k, v = qkv split                    # each [N, C]
    a = softmax(q @ k.T / sqrt(C))         # [N, N]
    o = a @ v                              # [N, C]
    o2 = o.swapaxes(0,1).reshape(N, C)     # torch-faithful permutation
    out = o2 @ w_proj.T + b_proj           # [N, C]

Sharding: batch B=8 data-parallel across the 8 NeuronCores, no collectives.

Layout strategy (zero on-device transposes; host pre-transposes weights/x):
  - q.k fold:  a[n,m] = x_n^T (W_q^T W_k) x_m, so the device never computes
    q or k. Host passes hT = (W_q^T W_k)^T = W_k^T W_q in bf16; the device
    computes z = hT.T @ xT  ([c,m] layout, 1/3 the cost of qT+kT), then
    aT[m,n] = z[:,m].T @ xT[:,n].
  - v computed in [m,c] layout:      v[m,c]  = xT[:,m].T @ wvT
  - p = exp(aT/32 - ln 32) (global 1/32 downscale cancels in the softmax
    denominator; keeps p inside fp8-e4m3 range for the fp8 chunks)
  - o in [n,c] layout:               o[n,c]  = p[:,n].T @ v
    with v augmented by a ones column so rowsum(p) lands in [n,1] per-partition
  - the torch permutation satisfies out[2t+s, d] = sum_c2 o[1024s+c2, t] *
    wprojT[c2, d], i.e. proj is a plain matmul over o's partition axis in
    half-blocks; output rows written with a stride-2 row DMA.

Precision: bf16 storage (measured: fp16 runs 1.2x slower on the real PE). The first 2*F8_PAIRS m-chunks of the o matmul
run as fp8-e4m3 DoubleRow pairs (2x PE throughput); quantization error scales
as sqrt(fraction) and is kept within the 2e-2 budget.

Startup: the z phase runs nb-outer / j-mid / cc-inner with cc-granular h/x
DMAs so the first matmul only waits on ~384KB instead of ~1MB.
"""

import numpy as np
import ml_dtypes

bf16h = ml_dtypes.bfloat16

B, N, C = 8, 2048, 1024
P = 128
NB = 512          # free-dim block for matmuls (one PSUM bank)
SCALE = 1.0 / 32.0
PBIAS = float(-np.log(32.0))  # global p downscale, cancels in softmax denom
F8_PAIRS = 3      # o-matmul m-chunk pairs (of 8) in fp8 DoubleRow
MF8 = 2 * F8_PAIRS  # fp8 m-tiles (the first MF8 of 16)


def _patch_tile_drain():
    """Walrus in this container rejects >~4 sem waits on one instruction; the
    TileContext exit drain aggregates one wait per active processor. Re-emit
    them as individual SP wait_ge instructions before the drain."""
    import concourse.tile as tile
    from concourse import mybir
    from concourse.vector_clock import ScopedClock

    if getattr(tile.TileContext, "_drain_patched", False):
        return

    def _drain_and_barrier(self, tick_clock, wait_clock):
        nc = self.nc
        probe = nc.sync.nop(nofuse=True)
        wait_clock.add_sem_waits(
            probe.ins, ScopedClock({None: tick_clock.global_clock})
        )
        si = probe.ins.sync_info
        waits = list(si.on_wait) if si is not None and si.on_wait else []
        probe.ins.sync_info = mybir.SyncInfo(
            on_wait=[],
            on_update=list(si.on_update) if si is not None and si.on_update else [],
        )
        handles = {h.num: h for h in self.sems.allocated().values()}
        for w in waits:
            assert w.wait_mode == "sem-ge-imm", w
            nc.sync.wait_ge(handles[w.id], w.wait_value)
        nc.sync.drain()
        nc.all_engine_barrier()
        popped = nc._tile_sem_poison_stack.pop()
        assert popped is self._sem_poison
        nc.clear_and_free_semaphores(list(self.sems.allocated().values()))
        nc.all_engine_barrier()

    tile.TileContext._drain_and_barrier = _drain_and_barrier
    tile.TileContext._drain_patched = True


def _split_excess_waits(nc, max_keep=1):
    """Walrus in this container rejects instructions with more than a couple
    of sem waits. Move excess waits onto single-wait EventSemaphore
    instructions inserted just before the offender on the same engine
    (engines execute their stream in order, so a chain of waits == one
    multi-wait)."""
    from concourse import mybir

    ctr = 0
    for f in nc.m.functions:
        for bb in f.blocks:
            il = list(bb.instructions)
            out = []
            changed = False
            for inst in il:
                si = inst.sync_info
                waits = list(si.on_wait) if si is not None and si.on_wait else []
                if len(waits) > max_keep:
                    changed = True
                    excess, keep = waits[:-max_keep], waits[-max_keep:]
                    for w in excess:
                        ev = mybir.InstEventSemaphore(
                            name=f"I-wsplit-{ctr}", ins=[], outs=[]
                        )
                        ctr += 1
                        ev.engine = inst.engine
                        ev.sync_info = mybir.SyncInfo(on_wait=[w], on_update=[])
                        out.append(ev)
                    inst.sync_info = mybir.SyncInfo(
                        on_wait=keep,
                        on_update=list(si.on_update) if si.on_update else [],
                    )
                out.append(inst)
            if changed:
                bb.instructions = out
    return nc


def _strip_init_barrier(nc):
    """Remove the Bass-init const-AP memsets and the initial all-engine
    barrier (block 0). The const tensors are never read by this kernel and
    each barrier round is self-zeroing, so later barriers are unaffected.
    Saves ~3us of kernel preamble."""
    from concourse import mybir

    bb = nc.m.functions[0].blocks[0]
    drop = (mybir.InstMemset, mybir.InstDrain, mybir.InstEventSemaphore)
    bb.instructions = [i for i in bb.instructions if not isinstance(i, drop)]
    return nc


def build_nc(split_waits=True):
    import concourse.bass as bass
    import concourse.tile as tile
    from concourse import mybir

    _patch_tile_drain()

    f16 = mybir.dt.bfloat16  # bf16: real PE runs fp16 1.2x slower than bf16
    e4 = mybir.dt.float8e4
    f32 = mybir.dt.float32
    DR = mybir.MatmulPerfMode.DoubleRow

    nc = bass.Bass()
    xT_ext = nc.declare_dram_parameter("xT", [C, N], f16, isOutput=False)
    hT_ext = nc.declare_dram_parameter("hT", [C, C], f16, isOutput=False)
    wvT_ext = nc.declare_dram_parameter("wvT", [C, C], f16, isOutput=False)
    wprojT_ext = nc.declare_dram_parameter("wprojT", [C, C], f16, isOutput=False)
    bias_ext = nc.declare_dram_parameter("bias", [P, C], f32, isOutput=False)
    out_ext = nc.declare_dram_parameter("out", [N, C], f32, isOutput=True)

    CC = C // P           # 8 contraction chunks over C
    MT = N // P           # 16 m-tiles
    NBLK = N // NB        # 4 n blocks
    CB = C // NB          # 2 c blocks
    M16 = MT - MF8        # fp16 m-tiles (the last M16)

    xT_r = xT_ext[:, :].rearrange("(cc p) n -> p cc n", p=P)
    hT_r = hT_ext[:, :].rearrange("(cc p) d -> p cc d", p=P)
    wvT_r = wvT_ext[:, :].rearrange("(cc p) d -> p cc d", p=P)
    wprojT_r = wprojT_ext[:, :].rearrange("(cc p) d -> p cc d", p=P)
    out_r = out_ext[:, :].rearrange("(t s) d -> t s d", s=2)

    with tile.TileContext(nc) as tc:
        with (
            tc.tile_pool(name="persist", bufs=1) as persist,
            tc.tile_pool(name="psum_main", bufs=6, space="PSUM") as psum_main,
        ):
            # ---- persistent SBUF tensors ----
            z_sb = persist.tile([P, CC, N], f16, tag="z")
            v16_sb = persist.tile([P, M16, C + 1], f16, tag="v16", name="v16_sb") if M16 else None
            v8_sb = persist.tile([P, MF8, C + 16], e4, tag="v8", name="v8_sb") if MF8 else None
            wprojT_sb = persist.tile([P, CC, C], f16, tag="wprojT")
            bias_sb = persist.tile([P, C], f32, tag="bias")
            # exp bias constant (activation bias must be an AP)
            pbias_sb = persist.tile([P, 1], f32, tag="pbias")
            nc.vector.memset(pbias_sb, PBIAS)

            # ones columns for the softmax denominator
            if v16_sb is not None:
                nc.vector.memset(v16_sb[:, :, C : C + 1], 1.0)
            if v8_sb is not None:
                nc.vector.memset(v8_sb[:, :, C : C + 1], 1.0)

            def v_dst(mt, cb):
                sl = slice(cb * NB, (cb + 1) * NB)
                if mt < MF8:
                    return v8_sb[:, mt, sl]
                return v16_sb[:, mt - MF8, sl]

            # xT stays resident through phase B (aT rhs); h/wv pool is freed
            # after phase A. nb-block 0 of x and all of h arrive cc-granular
            # so the first z matmul only waits on ~384KB.
            with tc.tile_pool(name="xpool", bufs=1) as xpool:
                x0_cc = [
                    xpool.tile([P, 1, NB], f16, tag=f"x0c_{cc}", name=f"x0c_{cc}")
                    for cc in range(CC)
                ]
                x_sb = [None] + [
                    xpool.tile([P, CC, NB], f16, tag=f"xsb_{nb}", name=f"xsb_{nb}")
                    for nb in range(1, NBLK)
                ]

                def x_ap(nb, cc):
                    if nb == 0:
                        return x0_cc[cc][:, 0, :]
                    return x_sb[nb][:, cc, :]

                # ---- phase A: z = hT.T @ xT and v = xT.T @ wvT ----
                with (
                    tc.tile_pool(name="wpool", bufs=1) as wpool,
                    tc.tile_pool(name="psz", bufs=2, space="PSUM") as psz,
                ):
                    h_cc = [
                        wpool.tile([P, 1, C], f16, tag=f"hc_{cc}", name=f"hc_{cc}")
                        for cc in range(CC)
                    ]
                    wv_sb = [
                        wpool.tile([P, CC, NB], f16, tag=f"wvsb_{k}", name=f"wvsb_{k}")
                        for k in range(CB)
                    ]

                    # PE warmup: the tensor engine p-state ramps to full
                    # clock only after ~3us of continuous execution. Burn the
                    # ramp on dummy matmuls while the first DMAs land.
                    warm = wpool.tile([P, P], f16, tag="warm", name="warm")
                    nc.vector.memset(warm, 0.0)
                    wps = psum_main.tile([P, NB], f32, tag="ps", name="pswarm")
                    for _ in range(28):
                        nc.tensor.matmul(
                            wps[:, 0:P], warm, warm, start=True, stop=True
                        )

                    # DMA order == consumption order; alternate SP/ACT rings.
                    # First the cc-interleaved h + x(nb0) pairs (x_sb[1]
                    # slotted mid-way so nb=1 never stalls), then the
                    # remaining x blocks, wv, and tail-only tensors.
                    for cc in range(CC):
                        nc.sync.dma_start(out=h_cc[cc], in_=hT_r[:, cc : cc + 1, :])
                        nc.scalar.dma_start(
                            out=x0_cc[cc], in_=xT_r[:, cc : cc + 1, 0:NB]
                        )
                        if cc == 4:
                            nc.sync.dma_start(
                                out=x_sb[1], in_=xT_r[:, :, NB : 2 * NB]
                            )
                    nc.sync.dma_start(out=x_sb[2], in_=xT_r[:, :, 2 * NB : 3 * NB])
                    nc.scalar.dma_start(out=wv_sb[0], in_=wvT_r[:, :, 0:NB])
                    nc.sync.dma_start(out=x_sb[3], in_=xT_r[:, :, 3 * NB : 4 * NB])
                    nc.scalar.dma_start(out=wv_sb[1], in_=wvT_r[:, :, NB : 2 * NB])
                    # needed only at the tail — load after the critical inputs
                    nc.scalar.dma_start(out=bias_sb, in_=bias_ext[:, :])
                    nc.sync.dma_start(out=wprojT_sb, in_=wprojT_r)

                    # z[j-tile, n-block] = hT[:, j].T @ xT[:, n]
                    # cc OUTER across 8 concurrently-open PSUM banks: the PE
                    # consumes h/x chunks in exact DMA arrival order, so the
                    # stream never stalls on a chunk that is still in flight.
                    for nb in range(NBLK):
                        zpsums = [
                            psum_main.tile([P, NB], f32, tag="ps", name=f"ps_z_{nb}_{j}")
                            for j in range(6)
                        ] + [
                            psz.tile([P, NB], f32, tag="psz", name=f"psz_{nb}_{j}")
                            for j in range(2)
                        ]
                        for cc in range(CC):
                            for j in range(CC):
                                nc.tensor.matmul(
                                    zpsums[j],
                                    h_cc[cc][:, 0, j * P : (j + 1) * P],
                                    x_ap(nb, cc),
                                    start=(cc == 0),
                                    stop=(cc == CC - 1),
                                )
                        for j in range(CC):
                            eng = nc.vector if j % 2 == 0 else nc.scalar
                            if j % 2 == 0:
                                nc.vector.tensor_copy(
                                    out=z_sb[:, j, nb * NB : (nb + 1) * NB],
                                    in_=zpsums[j],
                                )
                            else:
                                nc.scalar.activation(
                                    out=z_sb[:, j, nb * NB : (nb + 1) * NB],
                                    in_=zpsums[j],
                                    func=mybir.ActivationFunctionType.Copy,
                                )

                    # v[m-tile, c-block] = xT[:, m].T @ wvT[:, c]
                    for mt in range(MT):
                        psums = [
                            psum_main.tile([P, NB], f32, tag="ps", name=f"ps_v_{mt}_{i}")
                            for i in range(CB)
                        ]
                        for cc in range(CC):
                            if mt < 4:
                                lhsT = x0_cc[cc][:, 0, mt * P : (mt + 1) * P]
                            else:
                                lhsT = x_sb[mt // 4][:, cc, (mt % 4) * P : (mt % 4 + 1) * P]
                            for cb in range(CB):
                                nc.tensor.matmul(
                                    psums[cb],
                                    lhsT,
                                    wv_sb[cb][:, cc, :],
                                    start=(cc == 0),
                                    stop=(cc == CC - 1),
                                )
                        for cb in range(CB):
                            nc.scalar.activation(
                                out=v_dst(mt, cb),
                                in_=psums[cb],
                                func=mybir.ActivationFunctionType.Copy,
                            )

                # ---- phase B: attention, one 512-wide n-block at a time ----
                with (
                    tc.tile_pool(name="attn", bufs=1) as attn_pool,
                    tc.tile_pool(name="pT", bufs=1) as pT_pool,
                    tc.tile_pool(name="small", bufs=8) as small_pool,
                    tc.tile_pool(name="outbuf", bufs=4) as out_pool,
                    tc.tile_pool(name="psum_sum", bufs=2, space="PSUM") as psum_sum,
                ):
                    o_sb = attn_pool.tile([P, MT, C], f16, tag="o")

                    def emit_proj(s):
                        # out[2t+s, d] = sum_c2 o[1024s+c2, t] wprojT[c2, d]
                        for tt in range(CC):  # 8 t-tiles of 128 (t in [0,1024))
                            psums = [
                                psum_main.tile(
                                    [P, NB], f32, tag="ps", name=f"ps_p_{s}_{tt}_{i}"
                                )
                                for i in range(CB)
                            ]
                            for k in range(CC):
                                lhsT = o_sb[:, CC * s + k, tt * P : (tt + 1) * P]
                                for db in range(CB):
                                    nc.tensor.matmul(
                                        psums[db],
                                        lhsT,
                                        wprojT_sb[:, k, db * NB : (db + 1) * NB],
                                        start=(k == 0),
                                        stop=(k == CC - 1),
                                    )
                            # chunk the very last drain (2 halves per psum,
                            # DMAs spread over idle queues) for a short tail
                            nchunk = 2 if (s == 1 and tt == CC - 1) else 1
                            csz = NB // nchunk
                            dma_engs = [nc.sync, nc.scalar, nc.gpsimd, nc.sync]
                            di = 0
                            for db in range(CB):
                                for u in range(nchunk):
                                    outt = out_pool.tile(
                                        [P, csz], f32, tag=f"outt{nchunk}",
                                        name=f"outt_{s}_{tt}_{db}_{u}",
                                    )
                                    lo = db * NB + u * csz
                                    nc.vector.tensor_add(
                                        out=outt,
                                        in0=psums[db][:, u * csz : (u + 1) * csz],
                                        in1=bias_sb[:, lo : lo + csz],
                                    )
                                    eng = dma_engs[di % len(dma_engs)] if nchunk > 1 else nc.sync
                                    di += 1
                                    eng.dma_start(
                                        out=out_r[
                                            tt * P : (tt + 1) * P, s, lo : lo + csz
                                        ],
                                        in_=outt,
                                    )

                    SB = 2  # n-blocks per superblock: one aT weight load
                    #         (z m-slice) feeds SB matmuls
                    for sbk in range(NBLK // SB):
                        pT16 = (
                            pT_pool.tile([P, M16, SB * NB], f16, tag="pT16", name="pT16")
                            if M16
                            else None
                        )
                        pT8 = (
                            pT_pool.tile([P, MF8, SB * NB], e4, tag="pT8", name="pT8")
                            if MF8
                            else None
                        )

                        def p_dst(mt, u):
                            sl = slice(u * NB, (u + 1) * NB)
                            if mt < MF8:
                                return pT8[:, mt, sl]
                            return pT16[:, mt - MF8, sl]

                        # aT[m-tile, nblk] = z[:, m].T @ xT[:, nblk]
                        # p = exp(aT/32 - ln32)
                        for mt in range(MT):
                            apsums = [
                                psum_main.tile(
                                    [P, NB], f32, tag="ps", name=f"ps_a_{sbk}_{mt}_{u}"
                                )
                                for u in range(SB)
                            ]
                            for cc in range(CC):
                                lhsT = z_sb[:, cc, mt * P : (mt + 1) * P]
                                for u in range(SB):
                                    nc.tensor.matmul(
                                        apsums[u],
                                        lhsT,
                                        x_ap(sbk * SB + u, cc),
                                        start=(cc == 0),
                                        stop=(cc == CC - 1),
                                    )
                            for u in range(SB):
                                nc.scalar.activation(
                                    out=p_dst(mt, u),
                                    in_=apsums[u],
                                    func=mybir.ActivationFunctionType.Exp,
                                    scale=SCALE,
                                    bias=pbias_sb[:, :],
                                )
                        # o[n-tile, c] = p[:, n].T @ v  (+ ones column -> rowsum)
                        # fp8 m-pairs first (DoubleRow), then fp16 m-chunks.
                        for j in range(SB * NB // P):
                            nt = sbk * (SB * NB // P) + j
                            jsl = slice(j * P, (j + 1) * P)
                            opsums = [
                                psum_main.tile([P, NB], f32, tag="ps", name=f"ps_o_{nt}_{i}")
                                for i in range(CB)
                            ]
                            osum = psum_sum.tile([P, 1], f32, tag="ps_sum", name=f"ps_sum_{nt}")
                            nmm = F8_PAIRS + M16  # instructions per cb chain
                            idx = 0
                            # order per chunk: cb0, osum, cb1 — the 1-col osum
                            # runs under the already-loaded stationary, and the
                            # next weight load hides under the long cb1/cb0.
                            for q in range(F8_PAIRS):
                                psl = slice(2 * q, 2 * q + 2)
                                nc.tensor.matmul(
                                    opsums[0],
                                    pT8[:, psl, jsl],
                                    v8_sb[:, psl, 0:NB],
                                    start=(idx == 0),
                                    stop=(idx == nmm - 1),
                                    perf_mode=DR,
                                )
                                nc.tensor.matmul(
                                    osum,
                                    pT8[:, psl, jsl],
                                    v8_sb[:, psl, C : C + 1],
                                    start=(idx == 0),
                                    stop=(idx == nmm - 1),
                                    perf_mode=DR,
                                )
                                nc.tensor.matmul(
                                    opsums[1],
                                    pT8[:, psl, jsl],
                                    v8_sb[:, psl, NB : 2 * NB],
                                    start=(idx == 0),
                                    stop=(idx == nmm - 1),
                                    perf_mode=DR,
                                )
                                idx += 1
                            for mt in range(M16):
                                lhsT = pT16[:, mt, jsl]
                                nc.tensor.matmul(
                                    opsums[0],
                                    lhsT,
                                    v16_sb[:, mt, 0:NB],
                                    start=(idx == 0),
                                    stop=(idx == nmm - 1),
                                )
                                nc.tensor.matmul(
                                    osum,
                                    lhsT,
                                    v16_sb[:, mt, C : C + 1],
                                    start=(idx == 0),
                                    stop=(idx == nmm - 1),
                                )
                                nc.tensor.matmul(
                                    opsums[1],
                                    lhsT,
                                    v16_sb[:, mt, NB : 2 * NB],
                                    start=(idx == 0),
                                    stop=(idx == nmm - 1),
                                )
                                idx += 1
                            recip = small_pool.tile([P, 1], f32, tag="recip")
                            nc.vector.reciprocal(out=recip, in_=osum)
                            for cb in range(CB):
                                nc.vector.tensor_scalar_mul(
                                    out=o_sb[:, nt, cb * NB : (cb + 1) * NB],
                                    in0=opsums[cb],
                                    scalar1=recip,
                                )
                        # phase C half s=sbk: its o-tiles (nt 0..7 for s=0,
                        # 8..15 for s=1) are exactly this superblock's output,
                        # so the proj matmuls + output DMAs interleave here.
                        emit_proj(sbk)
    _strip_init_barrier(nc)
    if split_waits:
        _split_excess_waits(nc)
    return nc


_CACHED_NC = None


def _get_nc():
    global _CACHED_NC
    if _CACHED_NC is None:
        _CACHED_NC = build_nc()
    return _CACHED_NC


def _make_in_maps(x, w_qkv, w_proj, b_proj):
    x = np.asarray(x, dtype=np.float32)
    w_qkv = np.asarray(w_qkv, dtype=np.float32)
    w_proj = np.asarray(w_proj, dtype=np.float32)
    b_proj = np.asarray(b_proj, dtype=np.float32)

    w_q, w_k, w_v = w_qkv[0:C], w_qkv[C : 2 * C], w_qkv[2 * C : 3 * C]
    # hT = (W_q^T W_k)^T = W_k^T W_q, computed in f32 then rounded once
    hT = np.ascontiguousarray(w_k.T @ w_q).astype(bf16h)
    wvT = np.ascontiguousarray(w_v.T).astype(bf16h)
    wprojT = np.ascontiguousarray(w_proj.T).astype(bf16h)
    bias = np.ascontiguousarray(np.broadcast_to(b_proj, (P, C)))
    in_maps = []
    for b in range(B):
        xT = np.ascontiguousarray(x[b].T).astype(bf16h)
        in_maps.append(
            {"xT": xT, "hT": hT, "wvT": wvT, "wprojT": wprojT, "bias": bias}
        )
    return in_maps


def kernel(x, w_qkv, w_proj, b_proj):
    from concourse.bass_utils import run_bass_kernel_spmd

    nc = _get_nc()
    in_maps = _make_in_maps(x, w_qkv, w_proj, b_proj)
    res = run_bass_kernel_spmd(nc, in_maps, core_ids=list(range(B)))
    return np.stack([res.results[b]["out"] for b in range(B)], axis=0)


def kernel_traced(x, w_qkv, w_proj, b_proj, **trace_kwargs):
    """Like kernel() but with NTFF profiling; returns (out, BassKernelResults)."""
    from concourse.bass_utils import run_bass_kernel_spmd

    nc = _get_nc()
    in_maps = _make_in_maps(x, w_qkv, w_proj, b_proj)
    res = run_bass_kernel_spmd(
        nc, in_maps, core_ids=list(range(B)), trace=True, **trace_kwargs
    )
    out = np.stack([res.results[b]["out"] for b in range(B)], axis=0)
    return out, res


# revision 13
# speedup vs baseline: 1.2284x; 1.0044x over previous
"""Bass/Trainium2 kernel for the single-head dense attention block.

Reference computation (per batch element b of 8):
    qkv = x @ w_qkv.T                      # [N, 3C]
    q, k, v = qkv split                    # each [N, C]
    a = softmax(q @ k.T / sqrt(C))         # [N, N]
    o = a @ v                              # [N, C]
    o2 = o.swapaxes(0,1).reshape(N, C)     # torch-faithful permutation
    out = o2 @ w_proj.T + b_proj           # [N, C]

Sharding: batch B=8 data-parallel across the 8 NeuronCores, no collectives.

Layout strategy (zero on-device transposes; host pre-transposes weights/x):
  - q.k fold:  a[n,m] = x_n^T (W_q^T W_k) x_m, so the device never computes
    q or k. Host passes hT = (W_q^T W_k)^T = W_k^T W_q in bf16; the device
    computes z = hT.T @ xT  ([c,m] layout, 1/3 the cost of qT+kT), then
    aT[m,n] = z[:,m].T @ xT[:,n].
  - v computed in [m,c] layout:      v[m,c]  = xT[:,m].T @ wvT
  - p = exp(aT/32 - ln 32) (global 1/32 downscale cancels in the softmax
    denominator; keeps p inside fp8-e4m3 range for the fp8 chunks)
  - o in [n,c] layout:               o[n,c]  = p[:,n].T @ v
    with v augmented by a ones column so rowsum(p) lands in [n,1] per-partition
  - the torch permutation satisfies out[2t+s, d] = sum_c2 o[1024s+c2, t] *
    wprojT[c2, d], i.e. proj is a plain matmul over o's partition axis in
    half-blocks; output rows written with a stride-2 row DMA.

Precision: bf16 storage (measured: fp16 runs 1.2x slower on the real PE). The first 2*F8_PAIRS m-chunks of the o matmul
run as fp8-e4m3 DoubleRow pairs (2x PE throughput); quantization error scales
as sqrt(fraction) and is kept within the 2e-2 budget.

Startup: the z phase runs nb-outer / j-mid / cc-inner with cc-granular h/x
DMAs so the first matmul only waits on ~384KB instead of ~1MB.
"""

import numpy as np
import ml_dtypes

bf16h = ml_dtypes.bfloat16

B, N, C = 8, 2048, 1024
P = 128
NB = 512          # free-dim block for matmuls (one PSUM bank)
SCALE = 1.0 / 32.0
PBIAS = float(-np.log(32.0))  # global p downscale, cancels in softmax denom
F8_PAIRS = 4      # o-matmul m-chunk pairs (of 8) in fp8 DoubleRow
MF8 = 2 * F8_PAIRS  # fp8 m-tiles (the first MF8 of 16)


def _patch_tile_drain():
    """Walrus in this container rejects >~4 sem waits on one instruction; the
    TileContext exit drain aggregates one wait per active processor. Re-emit
    them as individual SP wait_ge instructions before the drain."""
    import concourse.tile as tile
    from concourse import mybir
    from concourse.vector_clock import ScopedClock

    if getattr(tile.TileContext, "_drain_patched", False):
        return

    def _drain_and_barrier(self, tick_clock, wait_clock):
        nc = self.nc
        probe = nc.sync.nop(nofuse=True)
        wait_clock.add_sem_waits(
            probe.ins, ScopedClock({None: tick_clock.global_clock})
        )
        si = probe.ins.sync_info
        waits = list(si.on_wait) if si is not None and si.on_wait else []
        probe.ins.sync_info = mybir.SyncInfo(
            on_wait=[],
            on_update=list(si.on_update) if si is not None and si.on_update else [],
        )
        handles = {h.num: h for h in self.sems.allocated().values()}
        for w in waits:
            assert w.wait_mode == "sem-ge-imm", w
            nc.sync.wait_ge(handles[w.id], w.wait_value)
        nc.sync.drain()
        nc.all_engine_barrier()
        popped = nc._tile_sem_poison_stack.pop()
        assert popped is self._sem_poison
        nc.clear_and_free_semaphores(list(self.sems.allocated().values()))
        nc.all_engine_barrier()

    tile.TileContext._drain_and_barrier = _drain_and_barrier
    tile.TileContext._drain_patched = True


def _split_excess_waits(nc, max_keep=1):
    """Walrus in this container rejects instructions with more than a couple
    of sem waits. Move excess waits onto single-wait EventSemaphore
    instructions inserted just before the offender on the same engine
    (engines execute their stream in order, so a chain of waits == one
    multi-wait)."""
    from concourse import mybir

    ctr = 0
    for f in nc.m.functions:
        for bb in f.blocks:
            il = list(bb.instructions)
            out = []
            changed = False
            for inst in il:
                si = inst.sync_info
                waits = list(si.on_wait) if si is not None and si.on_wait else []
                if len(waits) > max_keep:
                    changed = True
                    excess, keep = waits[:-max_keep], waits[-max_keep:]
                    for w in excess:
                        ev = mybir.InstEventSemaphore(
                            name=f"I-wsplit-{ctr}", ins=[], outs=[]
                        )
                        ctr += 1
                        ev.engine = inst.engine
                        ev.sync_info = mybir.SyncInfo(on_wait=[w], on_update=[])
                        out.append(ev)
                    inst.sync_info = mybir.SyncInfo(
                        on_wait=keep,
                        on_update=list(si.on_update) if si.on_update else [],
                    )
                out.append(inst)
            if changed:
                bb.instructions = out
    return nc


def _strip_init_barrier(nc):
    """Remove the Bass-init const-AP memsets and the initial all-engine
    barrier (block 0). The const tensors are never read by this kernel and
    each barrier round is self-zeroing, so later barriers are unaffected.
    Saves ~3us of kernel preamble."""
    from concourse import mybir

    bb = nc.m.functions[0].blocks[0]
    drop = (mybir.InstMemset, mybir.InstDrain, mybir.InstEventSemaphore)
    bb.instructions = [i for i in bb.instructions if not isinstance(i, drop)]
    return nc


def build_nc(split_waits=True):
    import concourse.bass as bass
    import concourse.tile as tile
    from concourse import mybir

    _patch_tile_drain()

    f16 = mybir.dt.bfloat16  # bf16: real PE runs fp16 1.2x slower than bf16
    e4 = mybir.dt.float8e4
    f32 = mybir.dt.float32
    DR = mybir.MatmulPerfMode.DoubleRow

    nc = bass.Bass()
    xT_ext = nc.declare_dram_parameter("xT", [C, N], f16, isOutput=False)
    hT_ext = nc.declare_dram_parameter("hT", [C, C], f16, isOutput=False)
    wvT_ext = nc.declare_dram_parameter("wvT", [C, C], f16, isOutput=False)
    wprojT_ext = nc.declare_dram_parameter("wprojT", [C, C], f16, isOutput=False)
    bias_ext = nc.declare_dram_parameter("bias", [P, C], f32, isOutput=False)
    out_ext = nc.declare_dram_parameter("out", [N, C], f32, isOutput=True)

    CC = C // P           # 8 contraction chunks over C
    MT = N // P           # 16 m-tiles
    NBLK = N // NB        # 4 n blocks
    CB = C // NB          # 2 c blocks
    M16 = MT - MF8        # fp16 m-tiles (the last M16)

    xT_r = xT_ext[:, :].rearrange("(cc p) n -> p cc n", p=P)
    hT_r = hT_ext[:, :].rearrange("(cc p) d -> p cc d", p=P)
    wvT_r = wvT_ext[:, :].rearrange("(cc p) d -> p cc d", p=P)
    wprojT_r = wprojT_ext[:, :].rearrange("(cc p) d -> p cc d", p=P)
    out_r = out_ext[:, :].rearrange("(t s) d -> t s d", s=2)

    with tile.TileContext(nc) as tc:
        with (
            tc.tile_pool(name="persist", bufs=1) as persist,
            tc.tile_pool(name="psum_main", bufs=6, space="PSUM") as psum_main,
        ):
            # ---- persistent SBUF tensors ----
            z_sb = persist.tile([P, CC, N], f16, tag="z")
            v16_sb = persist.tile([P, M16, C + 1], f16, tag="v16", name="v16_sb") if M16 else None
            v8_sb = persist.tile([P, MF8, C + 16], e4, tag="v8", name="v8_sb") if MF8 else None
            wprojT_sb = persist.tile([P, CC, C], f16, tag="wprojT")
            bias_sb = persist.tile([P, C], f32, tag="bias")
            # exp bias constant (activation bias must be an AP)
            pbias_sb = persist.tile([P, 1], f32, tag="pbias")
            nc.vector.memset(pbias_sb, PBIAS)

            # ones columns for the softmax denominator
            if v16_sb is not None:
                nc.vector.memset(v16_sb[:, :, C : C + 1], 1.0)
            if v8_sb is not None:
                nc.vector.memset(v8_sb[:, :, C : C + 1], 1.0)

            def v_dst(mt, cb):
                sl = slice(cb * NB, (cb + 1) * NB)
                if mt < MF8:
                    return v8_sb[:, mt, sl]
                return v16_sb[:, mt - MF8, sl]

            # xT stays resident through phase B (aT rhs); h/wv pool is freed
            # after phase A. nb-block 0 of x and all of h arrive cc-granular
            # so the first z matmul only waits on ~384KB.
            with tc.tile_pool(name="xpool", bufs=1) as xpool:
                x0_cc = [
                    xpool.tile([P, 1, NB], f16, tag=f"x0c_{cc}", name=f"x0c_{cc}")
                    for cc in range(CC)
                ]
                x_sb = [None] + [
                    xpool.tile([P, CC, NB], f16, tag=f"xsb_{nb}", name=f"xsb_{nb}")
                    for nb in range(1, NBLK)
                ]

                def x_ap(nb, cc):
                    if nb == 0:
                        return x0_cc[cc][:, 0, :]
                    return x_sb[nb][:, cc, :]

                # ---- phase A: z = hT.T @ xT and v = xT.T @ wvT ----
                with (
                    tc.tile_pool(name="wpool", bufs=1) as wpool,
                    tc.tile_pool(name="psz", bufs=2, space="PSUM") as psz,
                ):
                    h_cc = [
                        wpool.tile([P, 1, C], f16, tag=f"hc_{cc}", name=f"hc_{cc}")
                        for cc in range(CC)
                    ]
                    wv_sb = [
                        wpool.tile([P, CC, NB], f16, tag=f"wvsb_{k}", name=f"wvsb_{k}")
                        for k in range(CB)
                    ]

                    # PE warmup: the tensor engine p-state ramps to full
                    # clock only after ~3us of continuous execution. Burn the
                    # ramp on dummy matmuls while the first DMAs land.
                    warm = wpool.tile([P, P], f16, tag="warm", name="warm")
                    nc.vector.memset(warm, 0.0)
                    wps = psum_main.tile([P, NB], f32, tag="ps", name="pswarm")
                    for _ in range(32):
                        nc.tensor.matmul(
                            wps[:, 0:P], warm, warm, start=True, stop=True
                        )

                    # DMA order == consumption order; alternate SP/ACT rings.
                    # First the cc-interleaved h + x(nb0) pairs (x_sb[1]
                    # slotted mid-way so nb=1 never stalls), then the
                    # remaining x blocks, wv, and tail-only tensors.
                    for cc in range(CC):
                        if cc == 0:
                            nc.sync.dma_start(
                                out=h_cc[0][:, :, 0 : C // 2],
                                in_=hT_r[:, 0:1, 0 : C // 2],
                            )
                            nc.scalar.dma_start(
                                out=x0_cc[0], in_=xT_r[:, 0:1, 0:NB]
                            )
                            nc.sync.dma_start(
                                out=h_cc[0][:, :, C // 2 : C],
                                in_=hT_r[:, 0:1, C // 2 : C],
                            )
                            continue
                        nc.sync.dma_start(out=h_cc[cc], in_=hT_r[:, cc : cc + 1, :])
                        nc.scalar.dma_start(
                            out=x0_cc[cc], in_=xT_r[:, cc : cc + 1, 0:NB]
                        )
                        if cc == 4:
                            nc.sync.dma_start(
                                out=x_sb[1], in_=xT_r[:, :, NB : 2 * NB]
                            )
                    nc.sync.dma_start(out=x_sb[2], in_=xT_r[:, :, 2 * NB : 3 * NB])
                    nc.scalar.dma_start(out=wv_sb[0], in_=wvT_r[:, :, 0:NB])
                    nc.sync.dma_start(out=x_sb[3], in_=xT_r[:, :, 3 * NB : 4 * NB])
                    nc.scalar.dma_start(out=wv_sb[1], in_=wvT_r[:, :, NB : 2 * NB])
                    # needed only at the tail — load after the critical inputs
                    nc.scalar.dma_start(out=bias_sb, in_=bias_ext[:, :])
                    nc.sync.dma_start(out=wprojT_sb, in_=wprojT_r)

                    # z[j-tile, n-block] = hT[:, j].T @ xT[:, n]
                    # cc OUTER across 8 concurrently-open PSUM banks: the PE
                    # consumes h/x chunks in exact DMA arrival order, so the
                    # stream never stalls on a chunk that is still in flight.
                    for nb in range(NBLK):
                        zpsums = [
                            psum_main.tile([P, NB], f32, tag="ps", name=f"ps_z_{nb}_{j}")
                            for j in range(6)
                        ] + [
                            psz.tile([P, NB], f32, tag="psz", name=f"psz_{nb}_{j}")
                            for j in range(2)
                        ]
                        for cc in range(CC):
                            for j in range(CC):
                                nc.tensor.matmul(
                                    zpsums[j],
                                    h_cc[cc][:, 0, j * P : (j + 1) * P],
                                    x_ap(nb, cc),
                                    start=(cc == 0),
                                    stop=(cc == CC - 1),
                                )
                        for j in range(CC):
                            eng = nc.vector if j % 2 == 0 else nc.scalar
                            if j % 2 == 0:
                                nc.vector.tensor_copy(
                                    out=z_sb[:, j, nb * NB : (nb + 1) * NB],
                                    in_=zpsums[j],
                                )
                            else:
                                nc.scalar.activation(
                                    out=z_sb[:, j, nb * NB : (nb + 1) * NB],
                                    in_=zpsums[j],
                                    func=mybir.ActivationFunctionType.Copy,
                                )

                    # v[m-tile, c-block] = xT[:, m].T @ wvT[:, c]
                    for mt in range(MT):
                        psums = [
                            psum_main.tile([P, NB], f32, tag="ps", name=f"ps_v_{mt}_{i}")
                            for i in range(CB)
                        ]
                        for cc in range(CC):
                            if mt < 4:
                                lhsT = x0_cc[cc][:, 0, mt * P : (mt + 1) * P]
                            else:
                                lhsT = x_sb[mt // 4][:, cc, (mt % 4) * P : (mt % 4 + 1) * P]
                            for cb in range(CB):
                                nc.tensor.matmul(
                                    psums[cb],
                                    lhsT,
                                    wv_sb[cb][:, cc, :],
                                    start=(cc == 0),
                                    stop=(cc == CC - 1),
                                )
                        for cb in range(CB):
                            nc.scalar.activation(
                                out=v_dst(mt, cb),
                                in_=psums[cb],
                                func=mybir.ActivationFunctionType.Copy,
                            )

                # ---- phase B: attention, one 512-wide n-block at a time ----
                with (
                    tc.tile_pool(name="attn", bufs=1) as attn_pool,
                    tc.tile_pool(name="pT", bufs=1) as pT_pool,
                    tc.tile_pool(name="small", bufs=8) as small_pool,
                    tc.tile_pool(name="outbuf", bufs=4) as out_pool,
                    tc.tile_pool(name="psum_sum", bufs=2, space="PSUM") as psum_sum,
                ):
                    o_sb = attn_pool.tile([P, MT, C], f16, tag="o")

                    def emit_proj(s):
                        # out[2t+s, d] = sum_c2 o[1024s+c2, t] wprojT[c2, d]
                        for tt in range(CC):  # 8 t-tiles of 128 (t in [0,1024))
                            psums = [
                                psum_main.tile(
                                    [P, NB], f32, tag="ps", name=f"ps_p_{s}_{tt}_{i}"
                                )
                                for i in range(CB)
                            ]
                            for k in range(CC):
                                lhsT = o_sb[:, CC * s + k, tt * P : (tt + 1) * P]
                                for db in range(CB):
                                    nc.tensor.matmul(
                                        psums[db],
                                        lhsT,
                                        wprojT_sb[:, k, db * NB : (db + 1) * NB],
                                        start=(k == 0),
                                        stop=(k == CC - 1),
                                    )
                            # chunk the very last drain (2 halves per psum,
                            # DMAs spread over idle queues) for a short tail
                            nchunk = 2 if (s == 1 and tt == CC - 1) else 1
                            csz = NB // nchunk
                            dma_engs = [nc.sync, nc.scalar, nc.gpsimd, nc.sync]
                            di = 0
                            for db in range(CB):
                                for u in range(nchunk):
                                    outt = out_pool.tile(
                                        [P, csz], f32, tag=f"outt{nchunk}",
                                        name=f"outt_{s}_{tt}_{db}_{u}",
                                    )
                                    lo = db * NB + u * csz
                                    nc.vector.tensor_add(
                                        out=outt,
                                        in0=psums[db][:, u * csz : (u + 1) * csz],
                                        in1=bias_sb[:, lo : lo + csz],
                                    )
                                    eng = dma_engs[di % len(dma_engs)] if nchunk > 1 else nc.sync
                                    di += 1
                                    eng.dma_start(
                                        out=out_r[
                                            tt * P : (tt + 1) * P, s, lo : lo + csz
                                        ],
                                        in_=outt,
                                    )

                    SB = 2  # n-blocks per superblock: one aT weight load
                    #         (z m-slice) feeds SB matmuls
                    for sbk in range(NBLK // SB):
                        pT16 = (
                            pT_pool.tile([P, M16, SB * NB], f16, tag="pT16", name="pT16")
                            if M16
                            else None
                        )
                        pT8 = (
                            pT_pool.tile([P, MF8, SB * NB], e4, tag="pT8", name="pT8")
                            if MF8
                            else None
                        )

                        def p_dst(mt, u):
                            sl = slice(u * NB, (u + 1) * NB)
                            if mt < MF8:
                                return pT8[:, mt, sl]
                            return pT16[:, mt - MF8, sl]

                        # aT[m-tile, nblk] = z[:, m].T @ xT[:, nblk]
                        # p = exp(aT/32 - ln32)
                        for mt in range(MT):
                            apsums = [
                                psum_main.tile(
                                    [P, NB], f32, tag="ps", name=f"ps_a_{sbk}_{mt}_{u}"
                                )
                                for u in range(SB)
                            ]
                            for cc in range(CC):
                                lhsT = z_sb[:, cc, mt * P : (mt + 1) * P]
                                for u in range(SB):
                                    nc.tensor.matmul(
                                        apsums[u],
                                        lhsT,
                                        x_ap(sbk * SB + u, cc),
                                        start=(cc == 0),
                                        stop=(cc == CC - 1),
                                    )
                            for u in range(SB):
                                nc.scalar.activation(
                                    out=p_dst(mt, u),
                                    in_=apsums[u],
                                    func=mybir.ActivationFunctionType.Exp,
                                    scale=SCALE,
                                    bias=pbias_sb[:, :],
                                )
                        # o[n-tile, c] = p[:, n].T @ v  (+ ones column -> rowsum)
                        # fp8 m-pairs first (DoubleRow), then fp16 m-chunks.
                        for j in range(SB * NB // P):
                            nt = sbk * (SB * NB // P) + j
                            jsl = slice(j * P, (j + 1) * P)
                            opsums = [
                                psum_main.tile([P, NB], f32, tag="ps", name=f"ps_o_{nt}_{i}")
                                for i in range(CB)
                            ]
                            osum = psum_sum.tile([P, 1], f32, tag="ps_sum", name=f"ps_sum_{nt}")
                            nmm = F8_PAIRS + M16  # instructions per cb chain
                            idx = 0
                            # order per chunk: cb0, osum, cb1 — the 1-col osum
                            # runs under the already-loaded stationary, and the
                            # next weight load hides under the long cb1/cb0.
                            for q in range(F8_PAIRS):
                                psl = slice(2 * q, 2 * q + 2)
                                nc.tensor.matmul(
                                    opsums[0],
                                    pT8[:, psl, jsl],
                                    v8_sb[:, psl, 0:NB],
                                    start=(idx == 0),
                                    stop=(idx == nmm - 1),
                                    perf_mode=DR,
                                )
                                nc.tensor.matmul(
                                    osum,
                                    pT8[:, psl, jsl],
                                    v8_sb[:, psl, C : C + 1],
                                    start=(idx == 0),
                                    stop=(idx == nmm - 1),
                                    perf_mode=DR,
                                )
                                nc.tensor.matmul(
                                    opsums[1],
                                    pT8[:, psl, jsl],
                                    v8_sb[:, psl, NB : 2 * NB],
                                    start=(idx == 0),
                                    stop=(idx == nmm - 1),
                                    perf_mode=DR,
                                )
                                idx += 1
                            for mt in range(M16):
                                lhsT = pT16[:, mt, jsl]
                                nc.tensor.matmul(
                                    opsums[0],
                                    lhsT,
                                    v16_sb[:, mt, 0:NB],
                                    start=(idx == 0),
                                    stop=(idx == nmm - 1),
                                )
                                nc.tensor.matmul(
                                    osum,
                                    lhsT,
                                    v16_sb[:, mt, C : C + 1],
                                    start=(idx == 0),
                                    stop=(idx == nmm - 1),
                                )
                                nc.tensor.matmul(
                                    opsums[1],
                                    lhsT,
                                    v16_sb[:, mt, NB : 2 * NB],
                                    start=(idx == 0),
                                    stop=(idx == nmm - 1),
                                )
                                idx += 1
                            recip = small_pool.tile([P, 1], f32, tag="recip")
                            nc.vector.reciprocal(out=recip, in_=osum)
                            for cb in range(CB):
                                nc.vector.tensor_scalar_mul(
                                    out=o_sb[:, nt, cb * NB : (cb + 1) * NB],
                                    in0=opsums[cb],
                                    scalar1=recip,
                                )
                        # phase C half s=sbk: its o-tiles (nt 0..7 for s=0,
                        # 8..15 for s=1) are exactly this superblock's output,
                        # so the proj matmuls + output DMAs interleave here.
                        emit_proj(sbk)
    _strip_init_barrier(nc)
    if split_waits:
        _split_excess_waits(nc)
    return nc


_CACHED_NC = None


def _get_nc():
    global _CACHED_NC
    if _CACHED_NC is None:
        _CACHED_NC = build_nc()
    return _CACHED_NC


def _make_in_maps(x, w_qkv, w_proj, b_proj):
    x = np.asarray(x, dtype=np.float32)
    w_qkv = np.asarray(w_qkv, dtype=np.float32)
    w_proj = np.asarray(w_proj, dtype=np.float32)
    b_proj = np.asarray(b_proj, dtype=np.float32)

    w_q, w_k, w_v = w_qkv[0:C], w_qkv[C : 2 * C], w_qkv[2 * C : 3 * C]
    # hT = (W_q^T W_k)^T = W_k^T W_q, computed in f32 then rounded once
    hT = np.ascontiguousarray(w_k.T @ w_q).astype(bf16h)
    wvT = np.ascontiguousarray(w_v.T).astype(bf16h)
    wprojT = np.ascontiguousarray(w_proj.T).astype(bf16h)
    bias = np.ascontiguousarray(np.broadcast_to(b_proj, (P, C)))
    in_maps = []
    for b in range(B):
        xT = np.ascontiguousarray(x[b].T).astype(bf16h)
        in_maps.append(
            {"xT": xT, "hT": hT, "wvT": wvT, "wprojT": wprojT, "bias": bias}
        )
    return in_maps


def kernel(x, w_qkv, w_proj, b_proj):
    from concourse.bass_utils import run_bass_kernel_spmd

    nc = _get_nc()
    in_maps = _make_in_maps(x, w_qkv, w_proj, b_proj)
    res = run_bass_kernel_spmd(nc, in_maps, core_ids=list(range(B)))
    return np.stack([res.results[b]["out"] for b in range(B)], axis=0)


def kernel_traced(x, w_qkv, w_proj, b_proj, **trace_kwargs):
    """Like kernel() but with NTFF profiling; returns (out, BassKernelResults)."""
    from concourse.bass_utils import run_bass_kernel_spmd

    nc = _get_nc()
    in_maps = _make_in_maps(x, w_qkv, w_proj, b_proj)
    res = run_bass_kernel_spmd(
        nc, in_maps, core_ids=list(range(B)), trace=True, **trace_kwargs
    )
    out = np.stack([res.results[b]["out"] for b in range(B)], axis=0)
    return out, res


# revision 14
# speedup vs baseline: 1.2303x; 1.0015x over previous
"""Bass/Trainium2 kernel for the single-head dense attention block.

Reference computation (per batch element b of 8):
    qkv = x @ w_qkv.T                      # [N, 3C]
    q, k, v = qkv split                    # each [N, C]
    a = softmax(q @ k.T / sqrt(C))         # [N, N]
    o = a @ v                              # [N, C]
    o2 = o.swapaxes(0,1).reshape(N, C)     # torch-faithful permutation
    out = o2 @ w_proj.T + b_proj           # [N, C]

Sharding: batch B=8 data-parallel across the 8 NeuronCores, no collectives.

Layout strategy (zero on-device transposes; host pre-transposes weights/x):
  - q.k fold:  a[n,m] = x_n^T (W_q^T W_k) x_m, so the device never computes
    q or k. Host passes hT = (W_q^T W_k)^T = W_k^T W_q in bf16; the device
    computes z = hT.T @ xT  ([c,m] layout, 1/3 the cost of qT+kT), then
    aT[m,n] = z[:,m].T @ xT[:,n].
  - v computed in [m,c] layout:      v[m,c]  = xT[:,m].T @ wvT
  - p = exp(aT/32 - ln 32) (global 1/32 downscale cancels in the softmax
    denominator; keeps p inside fp8-e4m3 range for the fp8 chunks)
  - o in [n,c] layout:               o[n,c]  = p[:,n].T @ v
    with v augmented by a ones column so rowsum(p) lands in [n,1] per-partition
  - the torch permutation satisfies out[2t+s, d] = sum_c2 o[1024s+c2, t] *
    wprojT[c2, d], i.e. proj is a plain matmul over o's partition axis in
    half-blocks; output rows written with a stride-2 row DMA.

Precision: bf16 storage (measured: fp16 runs 1.2x slower on the real PE). The first 2*F8_PAIRS m-chunks of the o matmul
run as fp8-e4m3 DoubleRow pairs (2x PE throughput); quantization error scales
as sqrt(fraction) and is kept within the 2e-2 budget.

Startup: the z phase runs nb-outer / j-mid / cc-inner with cc-granular h/x
DMAs so the first matmul only waits on ~384KB instead of ~1MB.
"""

import numpy as np
import ml_dtypes

bf16h = ml_dtypes.bfloat16

B, N, C = 8, 2048, 1024
P = 128
NB = 512          # free-dim block for matmuls (one PSUM bank)
SCALE = 1.0 / 32.0
PBIAS = float(-np.log(32.0))  # global p downscale, cancels in softmax denom
F8_PAIRS = 4      # o-matmul m-chunk pairs (of 8) in fp8 DoubleRow
MF8 = 2 * F8_PAIRS  # fp8 m-tiles (the first MF8 of 16)


def _patch_tile_drain():
    """Walrus in this container rejects >~4 sem waits on one instruction; the
    TileContext exit drain aggregates one wait per active processor. Re-emit
    them as individual SP wait_ge instructions before the drain."""
    import concourse.tile as tile
    from concourse import mybir
    from concourse.vector_clock import ScopedClock

    if getattr(tile.TileContext, "_drain_patched", False):
        return

    def _drain_and_barrier(self, tick_clock, wait_clock):
        nc = self.nc
        probe = nc.sync.nop(nofuse=True)
        wait_clock.add_sem_waits(
            probe.ins, ScopedClock({None: tick_clock.global_clock})
        )
        si = probe.ins.sync_info
        waits = list(si.on_wait) if si is not None and si.on_wait else []
        probe.ins.sync_info = mybir.SyncInfo(
            on_wait=[],
            on_update=list(si.on_update) if si is not None and si.on_update else [],
        )
        handles = {h.num: h for h in self.sems.allocated().values()}
        for w in waits:
            assert w.wait_mode == "sem-ge-imm", w
            nc.sync.wait_ge(handles[w.id], w.wait_value)
        nc.sync.drain()
        nc.all_engine_barrier()
        popped = nc._tile_sem_poison_stack.pop()
        assert popped is self._sem_poison
        nc.clear_and_free_semaphores(list(self.sems.allocated().values()))
        nc.all_engine_barrier()

    tile.TileContext._drain_and_barrier = _drain_and_barrier
    tile.TileContext._drain_patched = True


def _split_excess_waits(nc, max_keep=1):
    """Walrus in this container rejects instructions with more than a couple
    of sem waits. Move excess waits onto single-wait EventSemaphore
    instructions inserted just before the offender on the same engine
    (engines execute their stream in order, so a chain of waits == one
    multi-wait)."""
    from concourse import mybir

    ctr = 0
    for f in nc.m.functions:
        for bb in f.blocks:
            il = list(bb.instructions)
            out = []
            changed = False
            for inst in il:
                si = inst.sync_info
                waits = list(si.on_wait) if si is not None and si.on_wait else []
                if len(waits) > max_keep:
                    changed = True
                    excess, keep = waits[:-max_keep], waits[-max_keep:]
                    for w in excess:
                        ev = mybir.InstEventSemaphore(
                            name=f"I-wsplit-{ctr}", ins=[], outs=[]
                        )
                        ctr += 1
                        ev.engine = inst.engine
                        ev.sync_info = mybir.SyncInfo(on_wait=[w], on_update=[])
                        out.append(ev)
                    inst.sync_info = mybir.SyncInfo(
                        on_wait=keep,
                        on_update=list(si.on_update) if si.on_update else [],
                    )
                out.append(inst)
            if changed:
                bb.instructions = out
    return nc


def _strip_init_barrier(nc):
    """Remove the Bass-init const-AP memsets and the initial all-engine
    barrier (block 0). The const tensors are never read by this kernel and
    each barrier round is self-zeroing, so later barriers are unaffected.
    Saves ~3us of kernel preamble."""
    from concourse import mybir

    bb = nc.m.functions[0].blocks[0]
    drop = (mybir.InstMemset, mybir.InstDrain, mybir.InstEventSemaphore)
    bb.instructions = [i for i in bb.instructions if not isinstance(i, drop)]
    return nc


def build_nc(split_waits=True):
    import concourse.bass as bass
    import concourse.tile as tile
    from concourse import mybir

    _patch_tile_drain()

    f16 = mybir.dt.bfloat16  # bf16: real PE runs fp16 1.2x slower than bf16
    e4 = mybir.dt.float8e4
    f32 = mybir.dt.float32
    DR = mybir.MatmulPerfMode.DoubleRow

    nc = bass.Bass()
    xT_ext = nc.declare_dram_parameter("xT", [C, N], f16, isOutput=False)
    hT_ext = nc.declare_dram_parameter("hT", [C, C], f16, isOutput=False)
    wvT_ext = nc.declare_dram_parameter("wvT", [C, C], f16, isOutput=False)
    wprojT_ext = nc.declare_dram_parameter("wprojT", [C, C], f16, isOutput=False)
    bias_ext = nc.declare_dram_parameter("bias", [P, C], f32, isOutput=False)
    out_ext = nc.declare_dram_parameter("out", [N, C], f32, isOutput=True)

    CC = C // P           # 8 contraction chunks over C
    MT = N // P           # 16 m-tiles
    NBLK = N // NB        # 4 n blocks
    CB = C // NB          # 2 c blocks
    M16 = MT - MF8        # fp16 m-tiles (the last M16)

    xT_r = xT_ext[:, :].rearrange("(cc p) n -> p cc n", p=P)
    hT_r = hT_ext[:, :].rearrange("(cc p) d -> p cc d", p=P)
    wvT_r = wvT_ext[:, :].rearrange("(cc p) d -> p cc d", p=P)
    wprojT_r = wprojT_ext[:, :].rearrange("(cc p) d -> p cc d", p=P)
    out_r = out_ext[:, :].rearrange("(t s) d -> t s d", s=2)

    with tile.TileContext(nc) as tc:
        with (
            tc.tile_pool(name="persist", bufs=1) as persist,
            tc.tile_pool(name="psum_main", bufs=6, space="PSUM") as psum_main,
        ):
            # ---- persistent SBUF tensors ----
            z_sb = persist.tile([P, CC, N], f16, tag="z")
            v16_sb = persist.tile([P, M16, C + 1], f16, tag="v16", name="v16_sb") if M16 else None
            v8_sb = persist.tile([P, MF8, C + 16], e4, tag="v8", name="v8_sb") if MF8 else None
            wprojT_sb = persist.tile([P, CC, C], f16, tag="wprojT")
            bias_sb = persist.tile([P, C], f32, tag="bias")
            # exp bias constant (activation bias must be an AP)
            pbias_sb = persist.tile([P, 1], f32, tag="pbias")
            nc.vector.memset(pbias_sb, PBIAS)

            # ones columns for the softmax denominator
            if v16_sb is not None:
                nc.vector.memset(v16_sb[:, :, C : C + 1], 1.0)
            if v8_sb is not None:
                nc.vector.memset(v8_sb[:, :, C : C + 1], 1.0)

            def v_dst(mt, cb):
                sl = slice(cb * NB, (cb + 1) * NB)
                if mt < MF8:
                    return v8_sb[:, mt, sl]
                return v16_sb[:, mt - MF8, sl]

            # xT stays resident through phase B (aT rhs); h/wv pool is freed
            # after phase A. nb-block 0 of x and all of h arrive cc-granular
            # so the first z matmul only waits on ~384KB.
            with tc.tile_pool(name="xpool", bufs=1) as xpool:
                x0_cc = [
                    xpool.tile([P, 1, NB], f16, tag=f"x0c_{cc}", name=f"x0c_{cc}")
                    for cc in range(CC)
                ]
                x_sb = [None] + [
                    xpool.tile([P, CC, NB], f16, tag=f"xsb_{nb}", name=f"xsb_{nb}")
                    for nb in range(1, NBLK)
                ]

                def x_ap(nb, cc):
                    if nb == 0:
                        return x0_cc[cc][:, 0, :]
                    return x_sb[nb][:, cc, :]

                # ---- phase A: z = hT.T @ xT and v = xT.T @ wvT ----
                with (
                    tc.tile_pool(name="wpool", bufs=1) as wpool,
                    tc.tile_pool(name="psz", bufs=2, space="PSUM") as psz,
                ):
                    h_cc = [
                        wpool.tile([P, 1, C], f16, tag=f"hc_{cc}", name=f"hc_{cc}")
                        for cc in range(CC)
                    ]
                    wv_sb = [
                        wpool.tile([P, CC, NB], f16, tag=f"wvsb_{k}", name=f"wvsb_{k}")
                        for k in range(CB)
                    ]

                    # PE warmup: the tensor engine p-state ramps to full
                    # clock only after ~3us of continuous execution. Burn the
                    # ramp on dummy matmuls while the first DMAs land.
                    warm = wpool.tile([P, P], f16, tag="warm", name="warm")
                    nc.vector.memset(warm, 0.0)
                    wps = psum_main.tile([P, NB], f32, tag="ps", name="pswarm")
                    for _ in range(32):
                        nc.tensor.matmul(
                            wps[:, 0:P], warm, warm, start=True, stop=True
                        )

                    # DMA order == consumption order; alternate SP/ACT rings.
                    # First the cc-interleaved h + x(nb0) pairs (x_sb[1]
                    # slotted mid-way so nb=1 never stalls), then the
                    # remaining x blocks, wv, and tail-only tensors.
                    for cc in range(CC):
                        nc.sync.dma_start(out=h_cc[cc], in_=hT_r[:, cc : cc + 1, :])
                        nc.scalar.dma_start(
                            out=x0_cc[cc], in_=xT_r[:, cc : cc + 1, 0:NB]
                        )
                    nc.sync.dma_start(out=x_sb[1], in_=xT_r[:, :, NB : 2 * NB])
                    nc.scalar.dma_start(out=wv_sb[0], in_=wvT_r[:, :, 0:NB])
                    nc.sync.dma_start(out=x_sb[2], in_=xT_r[:, :, 2 * NB : 3 * NB])
                    nc.scalar.dma_start(out=wv_sb[1], in_=wvT_r[:, :, NB : 2 * NB])
                    nc.sync.dma_start(out=x_sb[3], in_=xT_r[:, :, 3 * NB : 4 * NB])
                    # needed only at the tail — load after the critical inputs
                    nc.scalar.dma_start(out=bias_sb, in_=bias_ext[:, :])
                    nc.sync.dma_start(out=wprojT_sb, in_=wprojT_r)

                    # z[j-tile, n-block] = hT[:, j].T @ xT[:, n]
                    # cc OUTER across 8 concurrently-open PSUM banks: the PE
                    # consumes h/x chunks in exact DMA arrival order, so the
                    # stream never stalls on a chunk that is still in flight.
                    for nb in range(NBLK):
                        zpsums = [
                            psum_main.tile([P, NB], f32, tag="ps", name=f"ps_z_{nb}_{j}")
                            for j in range(6)
                        ] + [
                            psz.tile([P, NB], f32, tag="psz", name=f"psz_{nb}_{j}")
                            for j in range(2)
                        ]
                        for cc in range(CC):
                            for j in range(CC):
                                nc.tensor.matmul(
                                    zpsums[j],
                                    h_cc[cc][:, 0, j * P : (j + 1) * P],
                                    x_ap(nb, cc),
                                    start=(cc == 0),
                                    stop=(cc == CC - 1),
                                )
                        for j in range(CC):
                            eng = nc.vector if j % 2 == 0 else nc.scalar
                            if j % 2 == 0:
                                nc.vector.tensor_copy(
                                    out=z_sb[:, j, nb * NB : (nb + 1) * NB],
                                    in_=zpsums[j],
                                )
                            else:
                                nc.scalar.activation(
                                    out=z_sb[:, j, nb * NB : (nb + 1) * NB],
                                    in_=zpsums[j],
                                    func=mybir.ActivationFunctionType.Copy,
                                )

                    # v[m-tile, c-block] = xT[:, m].T @ wvT[:, c]
                    for mt in range(MT):
                        psums = [
                            psum_main.tile([P, NB], f32, tag="ps", name=f"ps_v_{mt}_{i}")
                            for i in range(CB)
                        ]
                        for cc in range(CC):
                            if mt < 4:
                                lhsT = x0_cc[cc][:, 0, mt * P : (mt + 1) * P]
                            else:
                                lhsT = x_sb[mt // 4][:, cc, (mt % 4) * P : (mt % 4 + 1) * P]
                            for cb in range(CB):
                                nc.tensor.matmul(
                                    psums[cb],
                                    lhsT,
                                    wv_sb[cb][:, cc, :],
                                    start=(cc == 0),
                                    stop=(cc == CC - 1),
                                )
                        for cb in range(CB):
                            nc.scalar.activation(
                                out=v_dst(mt, cb),
                                in_=psums[cb],
                                func=mybir.ActivationFunctionType.Copy,
                            )

                # ---- phase B: attention, one 512-wide n-block at a time ----
                with (
                    tc.tile_pool(name="attn", bufs=1) as attn_pool,
                    tc.tile_pool(name="pT", bufs=1) as pT_pool,
                    tc.tile_pool(name="small", bufs=8) as small_pool,
                    tc.tile_pool(name="outbuf", bufs=4) as out_pool,
                    tc.tile_pool(name="psum_sum", bufs=2, space="PSUM") as psum_sum,
                ):
                    o_sb = attn_pool.tile([P, MT, C], f16, tag="o")

                    def emit_proj(s):
                        # out[2t+s, d] = sum_c2 o[1024s+c2, t] wprojT[c2, d]
                        for tt in range(CC):  # 8 t-tiles of 128 (t in [0,1024))
                            psums = [
                                psum_main.tile(
                                    [P, NB], f32, tag="ps", name=f"ps_p_{s}_{tt}_{i}"
                                )
                                for i in range(CB)
                            ]
                            for k in range(CC):
                                lhsT = o_sb[:, CC * s + k, tt * P : (tt + 1) * P]
                                for db in range(CB):
                                    nc.tensor.matmul(
                                        psums[db],
                                        lhsT,
                                        wprojT_sb[:, k, db * NB : (db + 1) * NB],
                                        start=(k == 0),
                                        stop=(k == CC - 1),
                                    )
                            # chunk the very last drain (2 halves per psum,
                            # DMAs spread over idle queues) for a short tail
                            nchunk = 2 if (s == 1 and tt == CC - 1) else 1
                            csz = NB // nchunk
                            dma_engs = [nc.sync, nc.scalar, nc.gpsimd, nc.sync]
                            di = 0
                            for db in range(CB):
                                for u in range(nchunk):
                                    outt = out_pool.tile(
                                        [P, csz], f32, tag=f"outt{nchunk}",
                                        name=f"outt_{s}_{tt}_{db}_{u}",
                                    )
                                    lo = db * NB + u * csz
                                    nc.vector.tensor_add(
                                        out=outt,
                                        in0=psums[db][:, u * csz : (u + 1) * csz],
                                        in1=bias_sb[:, lo : lo + csz],
                                    )
                                    eng = dma_engs[di % len(dma_engs)] if nchunk > 1 else nc.sync
                                    di += 1
                                    eng.dma_start(
                                        out=out_r[
                                            tt * P : (tt + 1) * P, s, lo : lo + csz
                                        ],
                                        in_=outt,
                                    )

                    SB = 2  # n-blocks per superblock: one aT weight load
                    #         (z m-slice) feeds SB matmuls
                    for sbk in range(NBLK // SB):
                        pT16 = (
                            pT_pool.tile([P, M16, SB * NB], f16, tag="pT16", name="pT16")
                            if M16
                            else None
                        )
                        pT8 = (
                            pT_pool.tile([P, MF8, SB * NB], e4, tag="pT8", name="pT8")
                            if MF8
                            else None
                        )

                        def p_dst(mt, u):
                            sl = slice(u * NB, (u + 1) * NB)
                            if mt < MF8:
                                return pT8[:, mt, sl]
                            return pT16[:, mt - MF8, sl]

                        # aT[m-tile, nblk] = z[:, m].T @ xT[:, nblk]
                        # p = exp(aT/32 - ln32)
                        for mt in range(MT):
                            apsums = [
                                psum_main.tile(
                                    [P, NB], f32, tag="ps", name=f"ps_a_{sbk}_{mt}_{u}"
                                )
                                for u in range(SB)
                            ]
                            for cc in range(CC):
                                lhsT = z_sb[:, cc, mt * P : (mt + 1) * P]
                                for u in range(SB):
                                    nc.tensor.matmul(
                                        apsums[u],
                                        lhsT,
                                        x_ap(sbk * SB + u, cc),
                                        start=(cc == 0),
                                        stop=(cc == CC - 1),
                                    )
                            for u in range(SB):
                                nc.scalar.activation(
                                    out=p_dst(mt, u),
                                    in_=apsums[u],
                                    func=mybir.ActivationFunctionType.Exp,
                                    scale=SCALE,
                                    bias=pbias_sb[:, :],
                                )
                        # o[n-tile, c] = p[:, n].T @ v  (+ ones column -> rowsum)
                        # fp8 m-pairs first (DoubleRow), then fp16 m-chunks.
                        for j in range(SB * NB // P):
                            nt = sbk * (SB * NB // P) + j
                            jsl = slice(j * P, (j + 1) * P)
                            opsums = [
                                psum_main.tile([P, NB], f32, tag="ps", name=f"ps_o_{nt}_{i}")
                                for i in range(CB)
                            ]
                            osum = psum_sum.tile([P, 1], f32, tag="ps_sum", name=f"ps_sum_{nt}")
                            nmm = F8_PAIRS + M16  # instructions per cb chain
                            idx = 0
                            # order per chunk: cb0, osum, cb1 — the 1-col osum
                            # runs under the already-loaded stationary, and the
                            # next weight load hides under the long cb1/cb0.
                            for q in range(F8_PAIRS):
                                psl = slice(2 * q, 2 * q + 2)
                                nc.tensor.matmul(
                                    opsums[0],
                                    pT8[:, psl, jsl],
                                    v8_sb[:, psl, 0:NB],
                                    start=(idx == 0),
                                    stop=(idx == nmm - 1),
                                    perf_mode=DR,
                                )
                                nc.tensor.matmul(
                                    osum,
                                    pT8[:, psl, jsl],
                                    v8_sb[:, psl, C : C + 1],
                                    start=(idx == 0),
                                    stop=(idx == nmm - 1),
                                    perf_mode=DR,
                                )
                                nc.tensor.matmul(
                                    opsums[1],
                                    pT8[:, psl, jsl],
                                    v8_sb[:, psl, NB : 2 * NB],
                                    start=(idx == 0),
                                    stop=(idx == nmm - 1),
                                    perf_mode=DR,
                                )
                                idx += 1
                            for mt in range(M16):
                                lhsT = pT16[:, mt, jsl]
                                nc.tensor.matmul(
                                    opsums[0],
                                    lhsT,
                                    v16_sb[:, mt, 0:NB],
                                    start=(idx == 0),
                                    stop=(idx == nmm - 1),
                                )
                                nc.tensor.matmul(
                                    osum,
                                    lhsT,
                                    v16_sb[:, mt, C : C + 1],
                                    start=(idx == 0),
                                    stop=(idx == nmm - 1),
                                )
                                nc.tensor.matmul(
                                    opsums[1],
                                    lhsT,
                                    v16_sb[:, mt, NB : 2 * NB],
                                    start=(idx == 0),
                                    stop=(idx == nmm - 1),
                                )
                                idx += 1
                            recip = small_pool.tile([P, 1], f32, tag="recip")
                            nc.vector.reciprocal(out=recip, in_=osum)
                            for cb in range(CB):
                                nc.vector.tensor_scalar_mul(
                                    out=o_sb[:, nt, cb * NB : (cb + 1) * NB],
                                    in0=opsums[cb],
                                    scalar1=recip,
                                )
                        # phase C half s=sbk: its o-tiles (nt 0..7 for s=0,
                        # 8..15 for s=1) are exactly this superblock's output,
                        # so the proj matmuls + output DMAs interleave here.
                        emit_proj(sbk)
    _strip_init_barrier(nc)
    if split_waits:
        _split_excess_waits(nc)
    return nc


_CACHED_NC = None


def _get_nc():
    global _CACHED_NC
    if _CACHED_NC is None:
        _CACHED_NC = build_nc()
    return _CACHED_NC


def _make_in_maps(x, w_qkv, w_proj, b_proj):
    x = np.asarray(x, dtype=np.float32)
    w_qkv = np.asarray(w_qkv, dtype=np.float32)
    w_proj = np.asarray(w_proj, dtype=np.float32)
    b_proj = np.asarray(b_proj, dtype=np.float32)

    w_q, w_k, w_v = w_qkv[0:C], w_qkv[C : 2 * C], w_qkv[2 * C : 3 * C]
    # hT = (W_q^T W_k)^T = W_k^T W_q, computed in f32 then rounded once
    hT = np.ascontiguousarray(w_k.T @ w_q).astype(bf16h)
    wvT = np.ascontiguousarray(w_v.T).astype(bf16h)
    wprojT = np.ascontiguousarray(w_proj.T).astype(bf16h)
    bias = np.ascontiguousarray(np.broadcast_to(b_proj, (P, C)))
    in_maps = []
    for b in range(B):
        xT = np.ascontiguousarray(x[b].T).astype(bf16h)
        in_maps.append(
            {"xT": xT, "hT": hT, "wvT": wvT, "wprojT": wprojT, "bias": bias}
        )
    return in_maps


def kernel(x, w_qkv, w_proj, b_proj):
    from concourse.bass_utils import run_bass_kernel_spmd

    nc = _get_nc()
    in_maps = _make_in_maps(x, w_qkv, w_proj, b_proj)
    res = run_bass_kernel_spmd(nc, in_maps, core_ids=list(range(B)))
    return np.stack([res.results[b]["out"] for b in range(B)], axis=0)


def kernel_traced(x, w_qkv, w_proj, b_proj, **trace_kwargs):
    """Like kernel() but with NTFF profiling; returns (out, BassKernelResults)."""
    from concourse.bass_utils import run_bass_kernel_spmd

    nc = _get_nc()
    in_maps = _make_in_maps(x, w_qkv, w_proj, b_proj)
    res = run_bass_kernel_spmd(
        nc, in_maps, core_ids=list(range(B)), trace=True, **trace_kwargs
    )
    out = np.stack([res.results[b]["out"] for b in range(B)], axis=0)
    return out, res
